# revision 1
# baseline (speedup 1.0000x reference)
"""Trainium2 Bass kernel for EnhancedCondConv2d (moe_routing).

Data-parallel over batch: 8 cores x 2 samples each. Full inputs in,
full outputs back.

Per-core program (per sample):
  1. routing: avgpool(x) -> tiny MLP -> softmax -> rweights [16]
  2. w[b] = sum_e rweights[e] * experts[e]  (block-diag PE matmuls)
  3. 3x3 grouped conv as 9 PSUM-accumulated shifted matmuls (float32r)
  4. SE: channel mean folded into PSUM eviction (ACT accum), MLP -> cw,
     in-place scale pass out *= cw
  5. CBAM: PE transposes -> DVE max / DVE sum over channels -> 7x7 conv
     as 14 banded-Toeplitz matmuls -> sigmoid -> sw
  6. final: out * sw_broadcast + x, DMA out
"""

import math
from contextlib import ExitStack

import numpy as np

import concourse.bass as bass
import concourse.bacc as bacc
import concourse.mybir as mybir
import concourse.tile as tile
from concourse.bass_utils import run_bass_kernel_spmd

F32 = mybir.dt.float32
F32R = mybir.dt.float32r
BF16 = mybir.dt.bfloat16
AX = mybir.AxisListType
ALU = mybir.AluOpType
ACTF = mybir.ActivationFunctionType

B, CI, CO, H, W, E, KK, RR = 16, 128, 128, 128, 128, 16, 3, 8
NCORES = 8
BL = B // NCORES  # 2 samples per core
EPS = 1e-5
HW = H * W
IKK = CI * KK * KK  # 1152
BNS = 1.0 / math.sqrt(1.0 + EPS)

_CACHE = {}


def _build_module():
    nc = bacc.Bacc("TRN2", target_bir_lowering=False, debug=False)

    # ---- external inputs (host-prepped layouts) ----
    x_d = nc.dram_tensor("x2", [BL, CI, H, W], F32, kind="ExternalInput").ap()
    xb_d = nc.dram_tensor("x2b", [BL, CI, H, W], BF16, kind="ExternalInput").ap()
    ew_d = nc.dram_tensor("experts_w", [16, 128, IKK], BF16, kind="ExternalInput").ap()
    wid_d = nc.dram_tensor("wident", [128, 134], F32, kind="ExternalInput").ap()
    rw1t_d = nc.dram_tensor("rw1t", [CI, 16], F32, kind="ExternalInput").ap()
    rw2t_d = nc.dram_tensor("rw2t", [16, CI], F32, kind="ExternalInput").ap()
    rw3t_d = nc.dram_tensor("rw3t", [CI, 16], F32, kind="ExternalInput").ap()
    caw1t_d = nc.dram_tensor("caw1t", [CO, 16], F32, kind="ExternalInput").ap()
    caw2t_d = nc.dram_tensor("caw2t", [16, CO], F32, kind="ExternalInput").ap()
    g1_d = nc.dram_tensor("rbn1_g", [16], F32, kind="ExternalInput").ap()
    b1_d = nc.dram_tensor("rbn1_b", [16], F32, kind="ExternalInput").ap()
    g2_d = nc.dram_tensor("rbn2_g", [CI], F32, kind="ExternalInput").ap()
    b2_d = nc.dram_tensor("rbn2_b", [CI], F32, kind="ExternalInput").ap()
    rb3_d = nc.dram_tensor("rb3", [E], F32, kind="ExternalInput").ap()
    cag1_d = nc.dram_tensor("ca_bn1_g", [16], F32, kind="ExternalInput").ap()
    cab1_d = nc.dram_tensor("ca_bn1_b", [16], F32, kind="ExternalInput").ap()
    cag2_d = nc.dram_tensor("ca_bn2_g", [CO], F32, kind="ExternalInput").ap()
    cab2_d = nc.dram_tensor("ca_bn2_b", [CO], F32, kind="ExternalInput").ap()
    saw_d = nc.dram_tensor("sawf", [98], F32, kind="ExternalInput").ap()
    sag_d = nc.dram_tensor("sa_bn_g", [1], F32, kind="ExternalInput").ap()
    sab_d = nc.dram_tensor("sa_bn_b", [1], F32, kind="ExternalInput").ap()
    bmask_d = nc.dram_tensor("bmask", [128, 8], BF16, kind="ExternalInput").ap()

    out_d = nc.dram_tensor("out", [BL, CO, H, W], F32, kind="ExternalOutput").ap()

    # internal DRAM scratch
    srw_d = nc.dram_tensor("scr_rw", [BL, E], F32).ap()
    ssw_d = nc.dram_tensor("scr_sw", [BL, H, W], BF16).ap()

    with tile.TileContext(nc) as tc, ExitStack() as ctx:
        _kernel_body(
            ctx, tc,
            x_d, xb_d, ew_d, wid_d, rw1t_d, rw2t_d, rw3t_d, caw1t_d, caw2t_d,
            g1_d, b1_d, g2_d, b2_d, rb3_d, cag1_d, cab1_d, cag2_d, cab2_d,
            saw_d, sag_d, sab_d, bmask_d, out_d, srw_d, ssw_d,
        )
    nc.compile()
    return nc


def _kernel_body(ctx, tc,
                 x_d, xb_d, ew_d, wid_d, rw1t_d, rw2t_d, rw3t_d, caw1t_d, caw2t_d,
                 g1_d, b1_d, g2_d, b2_d, rb3_d, cag1_d, cab1_d, cag2_d, cab2_d,
                 saw_d, sag_d, sab_d, bmask_d, out_d, srw_d, ssw_d):
    nc = tc.nc

    cpool = ctx.enter_context(tc.tile_pool(name="const", bufs=1))
    xpool = ctx.enter_context(tc.tile_pool(name="xp", bufs=1))
    opool = ctx.enter_context(tc.tile_pool(name="op", bufs=1))
    wpool = ctx.enter_context(tc.tile_pool(name="wp", bufs=1))
    epool = ctx.enter_context(tc.tile_pool(name="ep", bufs=3))
    spool = ctx.enter_context(tc.tile_pool(name="sp", bufs=1))
    fpool = ctx.enter_context(tc.tile_pool(name="fp", bufs=3))
    scpool = ctx.enter_context(tc.tile_pool(name="scr", bufs=2))

    pconv = ctx.enter_context(tc.tile_pool(name="pc", bufs=4, space="PSUM"))
    pw = ctx.enter_context(tc.tile_pool(name="pw", bufs=3, space="PSUM"))
    pr = ctx.enter_context(tc.tile_pool(name="prt", bufs=1, space="PSUM"))

    # ---------- constants ----------
    wident = cpool.tile([128, 134], F32, tag="wident")
    nc.sync.dma_start(wident, wid_d)
    ident = wident[:, 3:131]

    rw1t = cpool.tile([CI, 16], F32, tag="rw1t")
    nc.sync.dma_start(rw1t, rw1t_d)
    rw2t = cpool.tile([16, CI], F32, tag="rw2t")
    nc.sync.dma_start(rw2t, rw2t_d)
    rw3t = cpool.tile([CI, 16], F32, tag="rw3t")
    nc.sync.dma_start(rw3t, rw3t_d)
    caw1t = cpool.tile([CO, 16], F32, tag="caw1t")
    nc.sync.dma_start(caw1t, caw1t_d)
    caw2t = cpool.tile([16, CO], F32, tag="caw2t")
    nc.sync.dma_start(caw2t, caw2t_d)

    def vec_const(dst_tag, src_ap, n, scale):
        raw = cpool.tile([n, 1], F32, tag=dst_tag + "_r")
        nc.sync.dma_start(raw, src_ap.unsqueeze(1))
        out = cpool.tile([n, 1], F32, tag=dst_tag)
        nc.vector.tensor_scalar_mul(out, raw, float(scale))
        return out

    gs1 = vec_const("gs1", g1_d, 16, BNS / HW)
    bb1 = vec_const("bb1", b1_d, 16, 1.0)
    gs2 = vec_const("gs2", g2_d, CI, BNS)
    bb2 = vec_const("bb2", b2_d, CI, 1.0)
    gsca1 = vec_const("gsca1", cag1_d, 16, BNS / HW)
    bbca1 = vec_const("bbca1", cab1_d, 16, 1.0)
    gsca2 = vec_const("gsca2", cag2_d, CO, BNS)
    bbca2 = vec_const("bbca2", cab2_d, CO, 1.0)

    rb3r = cpool.tile([1, E], F32, tag="rb3r")
    nc.sync.dma_start(rb3r, rb3_d.unsqueeze(0))

    # spatial-attention 7x7 taps, broadcast to all partitions
    sabc = cpool.tile([128, 98], F32, tag="sabc")
    nc.sync.dma_start(sabc, saw_d.unsqueeze(0).partition_broadcast(128))
    sak = cpool.tile([128, 98], F32, tag="sak")
    # mean channel (c=0) carries the 1/CO normalization of the channel-mean
    nc.vector.tensor_scalar_mul(sak[:, 0:49], sabc[:, 0:49], 1.0 / CO)
    nc.vector.tensor_copy(sak[:, 49:98], sabc[:, 49:98])

    gssa = cpool.tile([128, 1], F32, tag="gssa")
    nc.sync.dma_start(gssa, sag_d.unsqueeze(0).partition_broadcast(128))
    nc.vector.tensor_scalar_mul(gssa, gssa, BNS)
    bssa = cpool.tile([128, 1], F32, tag="bssa")
    nc.sync.dma_start(bssa, sab_d.unsqueeze(0).partition_broadcast(128))
    bmask = cpool.tile([128, 8], BF16, tag="bmask")
    nc.sync.dma_start(bmask, bmask_d)

    # banded Toeplitz matrices M[c,dh][k, w] = sum_dw sak[c,dh,dw] * S_dw[k, w]
    mcdh = []
    msA = cpool.tile([128, 128], F32, tag="msA")
    msB = cpool.tile([128, 128], F32, tag="msB")
    for t in range(14):
        c, dh = t // 7, t % 7
        dst = cpool.tile([128, 128], F32, tag=f"mcdh{t}")
        mcdh.append(dst)
        chain = [msA, msB, msA, msB, msA, msB, dst]
        for dw in range(7):
            sidx = c * 49 + dh * 7 + dw
            sc = sak[:, sidx:sidx + 1]
            shift = wident[:, dw:dw + 128]
            if dw == 0:
                nc.vector.tensor_scalar_mul(chain[0], shift, sc)
            else:
                nc.vector.scalar_tensor_tensor(
                    chain[dw], shift, sc, chain[dw - 1], ALU.mult, ALU.add)

    # ---------- per-sample ----------
    for b in range(BL):
        # -- load x (padded) --
        xp = xpool.tile([128, H + 2, W + 2], BF16, tag="x_pad")
        nc.vector.memset(xp[:, 0, :], 0.0)
        nc.vector.memset(xp[:, H + 1, :], 0.0)
        nc.vector.memset(xp[:, 1:H + 1, 0], 0.0)
        nc.vector.memset(xp[:, 1:H + 1, W + 1], 0.0)
        nc.sync.dma_start(xp[:, 1:H + 1, 1:W + 1], xb_d[b])

        # -- avgpool (sum; mean folded into BN scale) --
        psum_a = spool.tile([128, 1], F32, tag="psum_a")
        nc.vector.tensor_reduce(psum_a, xp[:, 1:65, 1:W + 1], AX.XY, ALU.add)
        pparts = spool.tile([128, 16], F32, tag="pparts")
        for i in range(16):
            pscr = scpool.tile([128, 4, 128], F32, tag="pscr")
            nc.scalar.activation(
                pscr, xp[:, 65 + 4 * i:69 + 4 * i, 1:W + 1], ACTF.Copy,
                accum_out=pparts[:, i:i + 1])
        psum_b = spool.tile([128, 1], F32, tag="psum_b")
        nc.vector.tensor_reduce(psum_b, pparts, AX.X, ALU.add)
        psum_t = spool.tile([128, 1], F32, tag="psum_t")
        nc.vector.tensor_add(psum_t, psum_a, psum_b)

        # -- routing MLP --
        mm1 = pr.tile([16, 1], F32, tag="r")
        nc.tensor.matmul(mm1, rw1t, psum_t, start=True, stop=True)
        h1 = spool.tile([16, 1], F32, tag="h1")
        nc.scalar.activation(h1, mm1, ACTF.Relu, bias=bb1, scale=gs1)
        mm2 = pr.tile([128, 1], F32, tag="r")
        nc.tensor.matmul(mm2, rw2t, h1, start=True, stop=True)
        gg = spool.tile([128, 1], F32, tag="gg")
        nc.scalar.activation(gg, mm2, ACTF.Sigmoid, bias=bb2, scale=gs2)
        mm3 = pr.tile([1, E], F32, tag="r")
        nc.tensor.matmul(mm3, gg, rw3t, start=True, stop=True)
        lg = spool.tile([1, E], F32, tag="lg")
        nc.vector.tensor_add(lg, mm3, rb3r)
        mx = spool.tile([1, 1], F32, tag="mx")
        nc.vector.tensor_reduce(mx, lg, AX.X, ALU.max)
        mxn = spool.tile([1, 1], F32, tag="mxn")
        nc.vector.tensor_scalar_mul(mxn, mx, -1.0)
        e16 = spool.tile([1, E], F32, tag="e16")
        nc.scalar.activation(e16, lg, ACTF.Exp, bias=mxn, scale=1.0)
        s1 = spool.tile([1, 1], F32, tag="s1")
        nc.vector.tensor_reduce(s1, e16, AX.X, ALU.add)
        rinv = spool.tile([1, 1], F32, tag="rinv")
        nc.vector.reciprocal(rinv, s1)
        rwrow = spool.tile([1, E], F32, tag="rwrow")
        nc.vector.tensor_scalar_mul(rwrow, e16, rinv)
        nc.sync.dma_start(srw_d[b].unsqueeze(0), rwrow)

        # block-diag routing weights [ (j,e)=128, j'=8 ]
        rwcol = spool.tile([128, 1], F32, tag="rwcol")
        nc.sync.dma_start(
            rwcol, srw_d[b].unsqueeze(0).broadcast_to([8, E]))
        rwblk = spool.tile([128, 8], BF16, tag="rwblk")
        nc.vector.tensor_scalar_mul(rwblk, bmask, rwcol)

        # -- w generation: w[i, k, o] = sum_e rw[e] experts[e, o, i, k] --
        wsb = wpool.tile([128, KK * KK, CO], BF16, tag="wsb")
        pwt = [pw.tile([128, 384], F32, tag="w", name=f"pw{b}_{i}") for i in range(3)]
        for og in range(16):
            ec = epool.tile([128, IKK], BF16, tag="echunk")
            nc.sync.dma_start(ec, ew_d[og])
            eck = ec.rearrange("p (i k) -> p k i", k=9)
            for k in range(9):
                lhs = eck[:, k, :]
                dst = pwt[k // 3][:, (k % 3) * 128 + og * 8:(k % 3) * 128 + og * 8 + 8]
                nc.tensor.matmul(dst, lhs, rwblk,
                                 start=True, stop=True)
        for k in range(9):
            nc.vector.tensor_copy(
                wsb[:, k, :], pwt[k // 3][:, (k % 3) * 128:(k % 3) * 128 + 128])

        # -- conv: 8 supers x 4 groups x 9 taps --
        osb = opool.tile([128, H, W], F32, tag="out_sb")
        cparts = spool.tile([128, 32], F32, tag="cparts")
        for sup in range(8):
            pcs = [pconv.tile([128, 512], F32, tag="c", name=f"pc{b}_{sup}_{i}")
                   for i in range(4)]
            for k in range(9):
                kh, kw = k // 3, k % 3
                lhs = wsb[:, k, :]
                for g in range(4):
                    r0 = sup * 16 + g * 4 + kh
                    rhs = xp[:, r0:r0 + 4, kw:kw + W]
                    nc.tensor.matmul(pcs[g], lhs, rhs,
                                     start=(k == 0), stop=(k == 8))
            for g in range(4):
                hr = sup * 16 + g * 4
                nc.scalar.activation(
                    osb[:, hr:hr + 4, :], pcs[g].rearrange("p (a b) -> p a b", a=4),
                    ACTF.Copy, accum_out=cparts[:, sup * 4 + g:sup * 4 + g + 1])

        # -- SE --
        cps = spool.tile([128, 1], F32, tag="cps")
        nc.vector.tensor_reduce(cps, cparts, AX.X, ALU.add)
        se1 = pr.tile([16, 1], F32, tag="r")
        nc.tensor.matmul(se1, caw1t, cps, start=True, stop=True)
        ch = spool.tile([16, 1], F32, tag="ch")
        nc.scalar.activation(ch, se1, ACTF.Relu, bias=bbca1, scale=gsca1)
        se2 = pr.tile([128, 1], F32, tag="r")
        nc.tensor.matmul(se2, caw2t, ch, start=True, stop=True)
        cw = spool.tile([128, 1], F32, tag="cw")
        nc.scalar.activation(cw, se2, ACTF.Sigmoid, bias=bbca2, scale=gsca2)

        # in-place SE scale of conv output
        for g in range(32):
            nc.scalar.mul(osb[:, 4 * g:4 * g + 4, :], osb[:, 4 * g:4 * g + 4, :], cw)

        # -- CBAM stats: transpose chunks, reduce over channels --
        spmax = spool.tile([128, 134], F32, tag="spmax")
        spsum = spool.tile([128, 134], F32, tag="spsum")
        nc.vector.memset(spmax[:, 0:3], 0.0)
        nc.vector.memset(spmax[:, 131:134], 0.0)
        nc.vector.memset(spsum[:, 0:3], 0.0)
        nc.vector.memset(spsum[:, 131:134], 0.0)
        for q in range(32):
            ptt = pconv.tile([128, 512], F32, tag="c")
            for j in range(4):
                chh = 4 * q + j
                nc.tensor.transpose(
                    ptt[:, 128 * j:128 * (j + 1)], osb[:, chh, :], ident)
            v = ptt.rearrange("p (a b) -> p a b", a=4)
            nc.vector.tensor_reduce(spmax[:, 3 + 4 * q:7 + 4 * q], v, AX.X, ALU.max)
            nc.vector.tensor_reduce(spsum[:, 3 + 4 * q:7 + 4 * q], v, AX.X, ALU.add)

        # -- CBAM 7x7 conv: 14 banded matmuls --
        psw = pconv.tile([128, 128], F32, tag="c")
        for t in range(14):
            c, dh = t // 7, t % 7
            src = spsum if c == 0 else spmax
            nc.tensor.matmul(psw, mcdh[t], src[:, dh:dh + 128],
                             start=(t == 0), stop=(t == 13))
        swT = spool.tile([128, 128], F32, tag="swT")
        nc.scalar.activation(swT, psw, ACTF.Sigmoid, bias=bssa, scale=gssa)
        pswh = pconv.tile([128, 128], F32, tag="c")
        nc.tensor.transpose(pswh, swT, ident)
        swH = spool.tile([128, 128], BF16, tag="swH")
        nc.vector.tensor_copy(swH, pswh)
        nc.sync.dma_start(ssw_d[b], swH)

        # -- final: out = out*sw + x --
        for g in range(32):
            swbc = fpool.tile([128, 4, 128], BF16, tag="swbc")
            nc.sync.dma_start(
                swbc, ssw_d[b, 4 * g:4 * g + 4, :].partition_broadcast(128))
            tmul = fpool.tile([128, 4, 128], F32, tag="tmul")
            nc.vector.tensor_mul(tmul, osb[:, 4 * g:4 * g + 4, :], swbc)
            xres = fpool.tile([128, 4, 128], F32, tag="xres")
            nc.sync.dma_start(xres, x_d[b, :, 4 * g:4 * g + 4, :])
            fo = fpool.tile([128, 4, 128], F32, tag="fo")
            eng = nc.vector if (g % 2 == 0) else nc.gpsimd
            eng.tensor_tensor(fo, tmul, xres, ALU.add)
            nc.sync.dma_start(out_d[b, :, 4 * g:4 * g + 4, :], fo)


def _host_prep(inp):
    import ml_dtypes
    experts = np.ascontiguousarray(inp["experts"], dtype=np.float32)
    ew = experts.reshape(E, CO, IKK).reshape(E, 16, 8, IKK)
    ew = np.ascontiguousarray(ew.transpose(1, 2, 0, 3)).reshape(16, 128, IKK)

    wid = np.zeros((128, 134), dtype=np.float32)
    wid[np.arange(128), np.arange(128) + 3] = 1.0

    sawf = np.ascontiguousarray(inp["sa_w"].reshape(2, 49)).reshape(98)

    bm = np.zeros((8, 16, 8), dtype=ml_dtypes.bfloat16)
    for j in range(8):
        bm[j, :, j] = 1.0
    bm = bm.reshape(128, 8)

    import ml_dtypes
    shared = {
        "experts_w": ew.astype(ml_dtypes.bfloat16),
        "wident": wid,
        "rw1t": np.ascontiguousarray(inp["rw1"].T, dtype=np.float32),
        "rw2t": np.ascontiguousarray(inp["rw2"].T, dtype=np.float32),
        "rw3t": np.ascontiguousarray(inp["rw3"].T, dtype=np.float32),
        "caw1t": np.ascontiguousarray(inp["ca_w1"].T, dtype=np.float32),
        "caw2t": np.ascontiguousarray(inp["ca_w2"].T, dtype=np.float32),
        "rbn1_g": np.asarray(inp["rbn1_g"], np.float32),
        "rbn1_b": np.asarray(inp["rbn1_b"], np.float32),
        "rbn2_g": np.asarray(inp["rbn2_g"], np.float32),
        "rbn2_b": np.asarray(inp["rbn2_b"], np.float32),
        "rb3": np.asarray(inp["rb3"], np.float32),
        "ca_bn1_g": np.asarray(inp["ca_bn1_g"], np.float32),
        "ca_bn1_b": np.asarray(inp["ca_bn1_b"], np.float32),
        "ca_bn2_g": np.asarray(inp["ca_bn2_g"], np.float32),
        "ca_bn2_b": np.asarray(inp["ca_bn2_b"], np.float32),
        "sawf": np.asarray(sawf, np.float32),
        "sa_bn_g": np.asarray(inp["sa_bn_g"], np.float32),
        "sa_bn_b": np.asarray(inp["sa_bn_b"], np.float32),
        "bmask": bm,
    }
    x = np.asarray(inp["x"], np.float32)
    in_maps = []
    for c in range(NCORES):
        m = dict(shared)
        xc = np.ascontiguousarray(x[BL * c:BL * (c + 1)])
        m["x2"] = xc
        m["x2b"] = xc.astype(ml_dtypes.bfloat16)
        in_maps.append(m)
    return in_maps


def get_module():
    if "nc" not in _CACHE:
        _CACHE["nc"] = _build_module()
    return _CACHE["nc"]


def kernel(**inputs):
    nc = get_module()
    in_maps = _host_prep(inputs)
    res = run_bass_kernel_spmd(nc, in_maps, core_ids=list(range(NCORES)))
    out = np.concatenate([r["out"] for r in res.results], axis=0)
    return out.astype(np.float32)



# revision 3
# speedup vs baseline: 1.3471x; 1.3471x over previous
"""Trainium2 Bass kernel for EnhancedCondConv2d (moe_routing).

Data-parallel over batch: 8 cores x 2 samples each. Full inputs in,
full outputs back.

Per-core program:
  1. routing (both samples): avgpool(x) -> tiny MLP -> softmax
  2. wgen (both samples fused): w[b] = sum_e rw[e]*experts[e] as 144
     block-diag matmuls with contiguous (FWL-friendly) expert layout
  3. conv per sample: 9 PSUM-accumulated shifted matmuls per 8-row
     strip, double-buffered PSUM so PE never stalls on eviction
  4. SE channel attention folded into PSUM eviction accumulators
  5. CBAM: channel-mean via cw-weighted ones-matmul on PE, channel-max
     via gpsimd partition_all_reduce; 7x7 conv as 14 banded-Toeplitz
     matmuls in [h-part, w-free] layout (no transposes anywhere)
  6. final: out = (cw*conv)*sw + x, all bf16, output bf16 (host casts)
"""

import math
from contextlib import ExitStack

import numpy as np

import concourse.bass as bass
import concourse.bacc as bacc
import concourse.mybir as mybir
import concourse.tile as tile
import concourse.bass_isa as bass_isa
from concourse.bass_utils import run_bass_kernel_spmd

F32 = mybir.dt.float32
BF16 = mybir.dt.bfloat16
AX = mybir.AxisListType
ALU = mybir.AluOpType
ACTF = mybir.ActivationFunctionType
RED = bass_isa.ReduceOp

B, CI, CO, H, W, E, RR = 16, 128, 128, 128, 128, 16, 8
NCORES = 8
BL = B // NCORES  # 2 samples per core
EPS = 1e-5
HW = H * W
BNS = 1.0 / math.sqrt(1.0 + EPS)

_CACHE = {}


def _build_module():
    nc = bacc.Bacc("TRN2", target_bir_lowering=False, debug=False)

    # ---- external inputs (host-prepped layouts) ----
    xp_d = nc.dram_tensor("xpad", [BL, CI, H + 2, W + 2], BF16,
                          kind="ExternalInput").ap()
    ew_d = nc.dram_tensor("experts_w", [16, 128, 9, 128], BF16,
                          kind="ExternalInput").ap()
    rw1t_d = nc.dram_tensor("rw1t", [CI, 16], F32, kind="ExternalInput").ap()
    rw2t_d = nc.dram_tensor("rw2t", [16, CI], F32, kind="ExternalInput").ap()
    rw3t_d = nc.dram_tensor("rw3t", [CI, 16], F32, kind="ExternalInput").ap()
    caw1t_d = nc.dram_tensor("caw1t", [CO, 16], F32, kind="ExternalInput").ap()
    caw2t_d = nc.dram_tensor("caw2t", [16, CO], F32, kind="ExternalInput").ap()
    gs1_d = nc.dram_tensor("gs1", [16], F32, kind="ExternalInput").ap()
    bb1_d = nc.dram_tensor("bb1", [16], F32, kind="ExternalInput").ap()
    gs2_d = nc.dram_tensor("gs2", [CI], F32, kind="ExternalInput").ap()
    bb2_d = nc.dram_tensor("bb2", [CI], F32, kind="ExternalInput").ap()
    rb3_d = nc.dram_tensor("rb3", [E], F32, kind="ExternalInput").ap()
    gsca1_d = nc.dram_tensor("gsca1", [16], F32, kind="ExternalInput").ap()
    bbca1_d = nc.dram_tensor("bbca1", [16], F32, kind="ExternalInput").ap()
    gsca2_d = nc.dram_tensor("gsca2", [CO], F32, kind="ExternalInput").ap()
    bbca2_d = nc.dram_tensor("bbca2", [CO], F32, kind="ExternalInput").ap()
    mcdh_d = nc.dram_tensor("mcdh", [128, 14, 128], BF16,
                            kind="ExternalInput").ap()
    gssa_d = nc.dram_tensor("gssa", [128], F32, kind="ExternalInput").ap()
    bssa_d = nc.dram_tensor("bssa", [128], F32, kind="ExternalInput").ap()
    bmask_d = nc.dram_tensor("bmask", [128, 8], BF16, kind="ExternalInput").ap()

    out_d = nc.dram_tensor("out", [BL, CO, H, W], BF16,
                           kind="ExternalOutput").ap()

    # internal DRAM scratch
    srw_d = nc.dram_tensor("scr_rw", [BL, E], F32).ap()
    ssw_d = nc.dram_tensor("scr_sw", [BL, HW], BF16).ap()

    with tile.TileContext(nc) as tc, ExitStack() as ctx:
        _kernel_body(ctx, tc, xp_d, ew_d, rw1t_d, rw2t_d, rw3t_d,
                     caw1t_d, caw2t_d, gs1_d, bb1_d, gs2_d, bb2_d, rb3_d,
                     gsca1_d, bbca1_d, gsca2_d, bbca2_d, mcdh_d, gssa_d,
                     bssa_d, bmask_d, out_d, srw_d, ssw_d)
    nc.compile()
    return nc


def _kernel_body(ctx, tc, xp_d, ew_d, rw1t_d, rw2t_d, rw3t_d,
                 caw1t_d, caw2t_d, gs1_d, bb1_d, gs2_d, bb2_d, rb3_d,
                 gsca1_d, bbca1_d, gsca2_d, bbca2_d, mcdh_d, gssa_d,
                 bssa_d, bmask_d, out_d, srw_d, ssw_d):
    nc = tc.nc

    cpool = ctx.enter_context(tc.tile_pool(name="const", bufs=1))
    xpool = ctx.enter_context(tc.tile_pool(name="xp", bufs=1))
    opool = ctx.enter_context(tc.tile_pool(name="ob", bufs=2))
    wpool = ctx.enter_context(tc.tile_pool(name="wp", bufs=1))
    epool = ctx.enter_context(tc.tile_pool(name="ep", bufs=2))
    spool = ctx.enter_context(tc.tile_pool(name="sp", bufs=2))
    fpool = ctx.enter_context(tc.tile_pool(name="fp", bufs=2))

    pc = ctx.enter_context(tc.tile_pool(name="pc", bufs=4, space="PSUM"))
    pm = ctx.enter_context(tc.tile_pool(name="pm", bufs=4, space="PSUM"))

    # ---------- constants ----------
    def cvec(tag, src, n):
        t = cpool.tile([n, 1], F32, tag=tag)
        nc.sync.dma_start(t, src.unsqueeze(1))
        return t

    rw1t = cpool.tile([CI, 16], F32, tag="rw1t")
    nc.sync.dma_start(rw1t, rw1t_d)
    rw2t = cpool.tile([16, CI], F32, tag="rw2t")
    nc.sync.dma_start(rw2t, rw2t_d)
    rw3t = cpool.tile([CI, 16], F32, tag="rw3t")
    nc.sync.dma_start(rw3t, rw3t_d)
    caw1t = cpool.tile([CO, 16], F32, tag="caw1t")
    nc.sync.dma_start(caw1t, caw1t_d)
    caw2t = cpool.tile([16, CO], F32, tag="caw2t")
    nc.sync.dma_start(caw2t, caw2t_d)
    gs1 = cvec("gs1", gs1_d, 16)
    bb1 = cvec("bb1", bb1_d, 16)
    gs2 = cvec("gs2", gs2_d, CI)
    bb2 = cvec("bb2", bb2_d, CI)
    gsca1 = cvec("gsca1", gsca1_d, 16)
    bbca1 = cvec("bbca1", bbca1_d, 16)
    gsca2 = cvec("gsca2", gsca2_d, CO)
    bbca2 = cvec("bbca2", bbca2_d, CO)
    gssa = cvec("gssa", gssa_d, 128)
    bssa = cvec("bssa", bssa_d, 128)
    rb3r = cpool.tile([1, E], F32, tag="rb3r")
    nc.sync.dma_start(rb3r, rb3_d.unsqueeze(0))
    bmask = cpool.tile([128, 8], BF16, tag="bmask")
    nc.sync.dma_start(bmask, bmask_d)
    mcdh = cpool.tile([128, 14, 128], BF16, tag="mcdh")
    nc.sync.dma_start(mcdh, mcdh_d)

    # ---------- input loads ----------
    xp = []
    for b in range(BL):
        t = xpool.tile([128, H + 2, W + 2], BF16, tag=f"xp{b}")
        nc.sync.dma_start(t, xp_d[b])
        xp.append(t)

    # ---------- routing (both samples) ----------
    rwcols = []
    for b in range(BL):
        psA = spool.tile([128, 1], F32, tag="psA")
        nc.vector.tensor_reduce(psA, xp[b][:, 0:80, :], AX.XY, ALU.add)
        pparts = spool.tile([128, 10], F32, tag="pparts")
        for i in range(10):
            pscr = fpool.tile([128, 5, W + 2], BF16, tag="pscr")
            nc.scalar.activation(
                pscr, xp[b][:, 80 + 5 * i:85 + 5 * i, :], ACTF.Copy,
                accum_out=pparts[:, i:i + 1])
        psB = spool.tile([128, 1], F32, tag="psB")
        nc.vector.tensor_reduce(psB, pparts, AX.X, ALU.add)
        psum_t = spool.tile([128, 1], F32, tag="psum_t")
        nc.vector.tensor_add(psum_t, psA, psB)

        mm1 = pm.tile([16, 1], F32, tag="m")
        nc.tensor.matmul(mm1, rw1t, psum_t, start=True, stop=True)
        h1 = spool.tile([16, 1], F32, tag="h1")
        nc.scalar.activation(h1, mm1, ACTF.Relu, bias=bb1, scale=gs1)
        mm2 = pm.tile([128, 1], F32, tag="m")
        nc.tensor.matmul(mm2, rw2t, h1, start=True, stop=True)
        gg = spool.tile([128, 1], F32, tag="gg")
        nc.scalar.activation(gg, mm2, ACTF.Sigmoid, bias=bb2, scale=gs2)
        mm3 = pm.tile([1, E], F32, tag="m")
        nc.tensor.matmul(mm3, gg, rw3t, start=True, stop=True)
        lg = spool.tile([1, E], F32, tag="lg")
        nc.vector.tensor_add(lg, mm3, rb3r)
        mx = spool.tile([1, 1], F32, tag="mx")
        nc.vector.tensor_reduce(mx, lg, AX.X, ALU.max)
        mxn = spool.tile([1, 1], F32, tag="mxn")
        nc.vector.tensor_scalar_mul(mxn, mx, -1.0)
        e16 = spool.tile([1, E], F32, tag="e16")
        nc.scalar.activation(e16, lg, ACTF.Exp, bias=mxn, scale=1.0)
        s1 = spool.tile([1, 1], F32, tag="s1")
        nc.vector.tensor_reduce(s1, e16, AX.X, ALU.add)
        rinv = spool.tile([1, 1], F32, tag="rinv")
        nc.vector.reciprocal(rinv, s1)
        rwrow = spool.tile([1, E], F32, tag="rwrow")
        nc.vector.tensor_scalar_mul(rwrow, e16, rinv)
        nc.sync.dma_start(srw_d[b].unsqueeze(0), rwrow)
        rwcol = spool.tile([128, 1], F32, tag=f"rwcol{b}")
        nc.sync.dma_start(rwcol, srw_d[b].unsqueeze(0).broadcast_to([8, E]))
        rwcols.append(rwcol)

    # block-diag routing weights for both samples [p=(o',e), j=(b,o')]
    rwblk = spool.tile([128, 2 * RR], BF16, tag="rwblk")
    nc.vector.tensor_scalar_mul(rwblk[:, 0:8], bmask, rwcols[0])
    nc.vector.tensor_scalar_mul(rwblk[:, 8:16], bmask, rwcols[1])

    # ---------- wgen (both samples) ----------
    wsb = [wpool.tile([128, 9, 128], BF16, tag=f"wsb{b}", name=f"wsb{b}")
           for b in range(BL)]
    for og in range(16):
        ec = epool.tile([128, 9, 128], BF16, tag="ec")
        nc.sync.dma_start(ec, ew_d[og])
        pw = pm.tile([128, 9, 16], F32, tag="m")
        for k in range(9):
            nc.tensor.matmul(pw[:, k, :], ec[:, k, :], rwblk,
                             start=True, stop=True)
        eng = nc.scalar if og % 2 == 0 else nc.vector
        if og % 2 == 0:
            nc.scalar.activation(wsb[0][:, :, og * 8:og * 8 + 8],
                                 pw[:, :, 0:8], ACTF.Copy)
            nc.vector.tensor_copy(wsb[1][:, :, og * 8:og * 8 + 8],
                                  pw[:, :, 8:16])
        else:
            nc.vector.tensor_copy(wsb[0][:, :, og * 8:og * 8 + 8],
                                  pw[:, :, 0:8])
            nc.scalar.activation(wsb[1][:, :, og * 8:og * 8 + 8],
                                 pw[:, :, 8:16], ACTF.Copy)

    # sp map tiles (shared across samples; pads written once)
    spsum_t = spool.tile([128, 134], BF16, tag="spsum_t")
    spmax_t = spool.tile([128, 134], BF16, tag="spmax_t")
    for t in (spsum_t, spmax_t):
        nc.vector.memset(t[:, 0:3], 0.0)
        nc.vector.memset(t[:, 131:134], 0.0)

    # ---------- per-sample: conv + SE + CBAM + final ----------
    for b in range(BL):
        osb = opool.tile([128, H, W], BF16, tag="osb")
        cparts = spool.tile([128, 32], F32, tag="cparts")

        # conv: 16 strips of 8 rows, 2 psum tiles each, PSUM double-buffered
        for sup in range(16):
            pcs = [pc.tile([128, 4, W], F32, tag="c", name=f"pc{b}_{sup}_{g}")
                   for g in range(2)]
            for k in range(9):
                kh, kw = divmod(k, 3)
                lhs = wsb[b][:, k, :]
                for g in range(2):
                    r0 = sup * 8 + g * 4 + kh
                    nc.tensor.matmul(pcs[g], lhs, xp[b][:, r0:r0 + 4, kw:kw + W],
                                     start=(k == 0), stop=(k == 8))
            for g in range(2):
                hr = sup * 8 + g * 4
                nc.scalar.activation(
                    osb[:, hr:hr + 4, :], pcs[g], ACTF.Copy,
                    accum_out=cparts[:, sup * 2 + g:sup * 2 + g + 1])

        # SE MLP
        cps = spool.tile([128, 1], F32, tag="cps")
        nc.vector.tensor_reduce(cps, cparts, AX.X, ALU.add)
        se1 = pm.tile([16, 1], F32, tag="m")
        nc.tensor.matmul(se1, caw1t, cps, start=True, stop=True)
        ch = spool.tile([16, 1], F32, tag="ch")
        nc.scalar.activation(ch, se1, ACTF.Relu, bias=bbca1, scale=gsca1)
        se2 = pm.tile([128, 1], F32, tag="m")
        nc.tensor.matmul(se2, caw2t, ch, start=True, stop=True)
        cw = spool.tile([128, 1], F32, tag="cw")
        nc.scalar.activation(cw, se2, ACTF.Sigmoid, bias=bbca2, scale=gsca2)
        cwb = spool.tile([128, 1], BF16, tag="cwb")
        nc.vector.tensor_copy(cwb, cw)

        # CBAM stats in 4 chunks of 32 rows:
        #   mean: cw-weighted channel sum on PE (M_sum carries the /128)
        #   then scale osb in place by cw, then channel max on gpsimd
        for c in range(4):
            mf = fpool.tile([1, 4096], BF16, tag="mf")
            for j in range(8):
                r0 = c * 32 + j * 4
                pmean = pm.tile([1, 512], F32, tag="m")
                nc.tensor.matmul(pmean, cwb, osb[:, r0:r0 + 4, :],
                                 start=True, stop=True)
                eng = nc.scalar if j % 2 == 0 else nc.vector
                if j % 2 == 0:
                    nc.scalar.activation(mf[:, j * 512:(j + 1) * 512], pmean,
                                         ACTF.Copy)
                else:
                    nc.vector.tensor_copy(mf[:, j * 512:(j + 1) * 512], pmean)
            nc.sync.dma_start(spsum_t[c * 32:(c + 1) * 32, 3:131], mf)

            nc.vector.tensor_scalar_mul(
                osb[:, c * 32:(c + 1) * 32, :],
                osb[:, c * 32:(c + 1) * 32, :], cw)
            tmx = fpool.tile([128, 4096], BF16, tag="tmx")
            nc.gpsimd.partition_all_reduce(
                tmx, osb[:, c * 32:(c + 1) * 32, :], 128, RED.max)
            nc.sync.dma_start(spmax_t[c * 32:(c + 1) * 32, 3:131], tmx[0:1, :])

        # CBAM 7x7 conv: 14 banded matmuls in [h, w] layout
        psw = pm.tile([128, 128], F32, tag="m")
        for t in range(14):
            cc, dw = divmod(t, 7)
            src = spsum_t if cc == 0 else spmax_t
            nc.tensor.matmul(psw, mcdh[:, t, :], src[:, dw:dw + 128],
                             start=(t == 0), stop=(t == 13))
        swsb = spool.tile([128, 128], BF16, tag="swsb")
        nc.scalar.activation(swsb, psw, ACTF.Sigmoid, bias=bssa, scale=gssa)
        nc.sync.dma_start(ssw_d[b].rearrange("(h w) -> h w", h=128), swsb)

        # final: out = osb*sw + x, chunked, all bf16
        for q in range(8):
            swbc = fpool.tile([128, 16, 128], BF16, tag="swbc")
            nc.sync.dma_start(
                swbc,
                ssw_d[b, q * 2048:(q + 1) * 2048].unsqueeze(0)
                .partition_broadcast(128))
            sl = osb[:, q * 16:(q + 1) * 16, :]
            nc.vector.tensor_mul(sl, sl, swbc)
            eng = nc.vector if q % 4 != 3 else nc.gpsimd
            eng.tensor_tensor(
                sl, sl, xp[b][:, 1 + q * 16:17 + q * 16, 1:W + 1], ALU.add)
            nc.sync.dma_start(out_d[b, :, q * 16:(q + 1) * 16, :], sl)


def _host_prep(inp):
    import ml_dtypes
    experts = np.ascontiguousarray(inp["experts"], dtype=np.float32)
    # [E, O, I, 3, 3] -> [og, p=(o'*16+e), k, i]
    ew = experts.reshape(E, 16, 8, CI, 9)          # [e, og, o', i, k]
    ew = ew.transpose(1, 2, 0, 4, 3)               # [og, o', e, k, i]
    ew = np.ascontiguousarray(ew).reshape(16, 128, 9, 128)

    bm = np.zeros((8, 16, 8), dtype=ml_dtypes.bfloat16)
    for j in range(8):
        bm[j, :, j] = 1.0
    bm = bm.reshape(128, 8)

    # banded Toeplitz for the 7x7 CBAM conv, [h, h'] per (c, dw)
    saw = np.asarray(inp["sa_w"], np.float32).reshape(2, 7, 7)
    M = np.zeros((14, 128, 128), dtype=np.float32)
    for c in range(2):
        scl = (1.0 / CO) if c == 0 else 1.0
        for dh in range(7):
            for dw in range(7):
                hp = np.arange(128)
                h = hp + dh - 3
                v = (h >= 0) & (h < 128)
                M[c * 7 + dw, h[v], hp[v]] += saw[c, dh, dw] * scl
    mcdh = np.ascontiguousarray(M.transpose(1, 0, 2))  # [h, t, h']

    x = np.asarray(inp["x"], np.float32)
    xpad = np.zeros((B, CI, H + 2, W + 2), dtype=ml_dtypes.bfloat16)
    xpad[:, :, 1:H + 1, 1:W + 1] = x

    shared = {
        "experts_w": ew.astype(ml_dtypes.bfloat16),
        "rw1t": np.ascontiguousarray(inp["rw1"].T, dtype=np.float32),
        "rw2t": np.ascontiguousarray(inp["rw2"].T, dtype=np.float32),
        "rw3t": np.ascontiguousarray(inp["rw3"].T, dtype=np.float32),
        "caw1t": np.ascontiguousarray(inp["ca_w1"].T, dtype=np.float32),
        "caw2t": np.ascontiguousarray(inp["ca_w2"].T, dtype=np.float32),
        "gs1": np.asarray(inp["rbn1_g"], np.float32) * (BNS / HW),
        "bb1": np.asarray(inp["rbn1_b"], np.float32),
        "gs2": np.asarray(inp["rbn2_g"], np.float32) * BNS,
        "bb2": np.asarray(inp["rbn2_b"], np.float32),
        "rb3": np.asarray(inp["rb3"], np.float32),
        "gsca1": np.asarray(inp["ca_bn1_g"], np.float32) * (BNS / HW),
        "bbca1": np.asarray(inp["ca_bn1_b"], np.float32),
        "gsca2": np.asarray(inp["ca_bn2_g"], np.float32) * BNS,
        "bbca2": np.asarray(inp["ca_bn2_b"], np.float32),
        "mcdh": mcdh.astype(ml_dtypes.bfloat16),
        "gssa": np.full(128, float(inp["sa_bn_g"][0]) * BNS, np.float32),
        "bssa": np.full(128, float(inp["sa_bn_b"][0]), np.float32),
        "bmask": bm,
    }
    in_maps = []
    for c in range(NCORES):
        m = dict(shared)
        m["xpad"] = np.ascontiguousarray(xpad[BL * c:BL * (c + 1)])
        in_maps.append(m)
    return in_maps


def get_module():
    if "nc" not in _CACHE:
        _CACHE["nc"] = _build_module()
    return _CACHE["nc"]


def kernel(**inputs):
    nc = get_module()
    in_maps = _host_prep(inputs)
    res = run_bass_kernel_spmd(nc, in_maps, core_ids=list(range(NCORES)))
    out = np.concatenate([r["out"] for r in res.results], axis=0)
    return out.astype(np.float32)


# revision 4
# speedup vs baseline: 1.7119x; 1.2707x over previous
"""Trainium2 Bass kernel for EnhancedCondConv2d (moe_routing).

Data-parallel over batch: 8 cores x 2 samples each. Full inputs in,
full outputs back.

Per-core program:
  1. routing (both samples): avgpool(x) -> tiny MLP -> softmax
  2. wgen (both samples fused): w[b] = sum_e rw[e]*experts[e] as 144
     block-diag matmuls with contiguous (FWL-friendly) expert layout
  3. conv per sample: 9 PSUM-accumulated shifted matmuls per 8-row
     strip, double-buffered PSUM so PE never stalls on eviction
  4. SE channel attention folded into PSUM eviction accumulators
  5. CBAM stats: cw-scaled transpose of each h-plane via one matmul
     against diag(cw) -> DVE max-reduce gives the channel max in
     [w-part, h] layout; channel mean via cw-weighted column matmul in
     flat [h, w] layout. 7x7 conv as banded-Toeplitz matmuls in both
     layouts, folded into one PSUM via an identity matmul.
  6. final: out = (osb*sw)*cw + x in place, all bf16 (host casts f32)
"""

import math
from contextlib import ExitStack

import numpy as np

import concourse.bass as bass
import concourse.bacc as bacc
import concourse.mybir as mybir
import concourse.tile as tile
from concourse.bass_utils import run_bass_kernel_spmd

F32 = mybir.dt.float32
BF16 = mybir.dt.bfloat16
AX = mybir.AxisListType
ALU = mybir.AluOpType
ACTF = mybir.ActivationFunctionType

B, CI, CO, H, W, E, RR = 16, 128, 128, 128, 128, 16, 8
NCORES = 8
BL = B // NCORES  # 2 samples per core
EPS = 1e-5
HW = H * W
BNS = 1.0 / math.sqrt(1.0 + EPS)

_CACHE = {}


def _build_module():
    nc = bacc.Bacc("TRN2", target_bir_lowering=False, debug=False)

    xp_d = nc.dram_tensor("xpad", [BL, CI, H + 2, W + 2], BF16,
                          kind="ExternalInput").ap()
    ew_d = nc.dram_tensor("experts_w", [16, 128, 9, 128], BF16,
                          kind="ExternalInput").ap()
    rw1t_d = nc.dram_tensor("rw1t", [CI, 16], F32, kind="ExternalInput").ap()
    rw2t_d = nc.dram_tensor("rw2t", [16, CI], F32, kind="ExternalInput").ap()
    rw3t_d = nc.dram_tensor("rw3t", [CI, 16], F32, kind="ExternalInput").ap()
    caw1t_d = nc.dram_tensor("caw1t", [CO, 16], F32, kind="ExternalInput").ap()
    caw2t_d = nc.dram_tensor("caw2t", [16, CO], F32, kind="ExternalInput").ap()
    gs1_d = nc.dram_tensor("gs1", [16], F32, kind="ExternalInput").ap()
    bb1_d = nc.dram_tensor("bb1", [16], F32, kind="ExternalInput").ap()
    gs2_d = nc.dram_tensor("gs2", [CI], F32, kind="ExternalInput").ap()
    bb2_d = nc.dram_tensor("bb2", [CI], F32, kind="ExternalInput").ap()
    rb3_d = nc.dram_tensor("rb3", [E], F32, kind="ExternalInput").ap()
    gsca1_d = nc.dram_tensor("gsca1", [16], F32, kind="ExternalInput").ap()
    bbca1_d = nc.dram_tensor("bbca1", [16], F32, kind="ExternalInput").ap()
    gsca2_d = nc.dram_tensor("gsca2", [CO], F32, kind="ExternalInput").ap()
    bbca2_d = nc.dram_tensor("bbca2", [CO], F32, kind="ExternalInput").ap()
    msum_d = nc.dram_tensor("msum", [128, 7, 128], BF16,
                            kind="ExternalInput").ap()
    mmax_d = nc.dram_tensor("mmax", [128, 7, 128], BF16,
                            kind="ExternalInput").ap()
    identb_d = nc.dram_tensor("identb", [128, 128], BF16,
                              kind="ExternalInput").ap()
    gssa_d = nc.dram_tensor("gssa", [128], F32, kind="ExternalInput").ap()
    bssa_d = nc.dram_tensor("bssa", [128], F32, kind="ExternalInput").ap()
    bmask_d = nc.dram_tensor("bmask", [128, 8], BF16, kind="ExternalInput").ap()

    out_d = nc.dram_tensor("out", [BL, CO, H, W], BF16,
                           kind="ExternalOutput").ap()

    srw_d = nc.dram_tensor("scr_rw", [BL, E], F32).ap()
    ssw_d = nc.dram_tensor("scr_sw", [BL, HW], BF16).ap()

    with tile.TileContext(nc) as tc, ExitStack() as ctx:
        _kernel_body(ctx, tc, xp_d, ew_d, rw1t_d, rw2t_d, rw3t_d,
                     caw1t_d, caw2t_d, gs1_d, bb1_d, gs2_d, bb2_d, rb3_d,
                     gsca1_d, bbca1_d, gsca2_d, bbca2_d, msum_d, mmax_d,
                     identb_d, gssa_d, bssa_d, bmask_d, out_d, srw_d, ssw_d)
    nc.compile()
    return nc


def _kernel_body(ctx, tc, xp_d, ew_d, rw1t_d, rw2t_d, rw3t_d,
                 caw1t_d, caw2t_d, gs1_d, bb1_d, gs2_d, bb2_d, rb3_d,
                 gsca1_d, bbca1_d, gsca2_d, bbca2_d, msum_d, mmax_d,
                 identb_d, gssa_d, bssa_d, bmask_d, out_d, srw_d, ssw_d):
    nc = tc.nc

    cpool = ctx.enter_context(tc.tile_pool(name="const", bufs=1))
    xpool = ctx.enter_context(tc.tile_pool(name="xp", bufs=1))
    opool = ctx.enter_context(tc.tile_pool(name="ob", bufs=2))
    wpool = ctx.enter_context(tc.tile_pool(name="wp", bufs=1))
    epool = ctx.enter_context(tc.tile_pool(name="ep", bufs=4))
    spool = ctx.enter_context(tc.tile_pool(name="sp", bufs=2))
    fpool = ctx.enter_context(tc.tile_pool(name="fp", bufs=2))

    pc = ctx.enter_context(tc.tile_pool(name="pc", bufs=4, space="PSUM"))
    pm = ctx.enter_context(tc.tile_pool(name="pm", bufs=4, space="PSUM"))

    # ---------- input loads first (engines are DMA-starved at head) ----------
    xp = []
    for b in range(BL):
        t = xpool.tile([128, H + 2, W + 2], BF16, tag=f"xp{b}", name=f"xp{b}")
        nc.sync.dma_start(t, xp_d[b])
        xp.append(t)

    # ---------- constants (issued on the scalar HWDGE queue) ----------
    def cvec(tag, src, n):
        t = cpool.tile([n, 1], F32, tag=tag, name=tag)
        nc.scalar.dma_start(t, src.unsqueeze(1))
        return t

    rw1t = cpool.tile([CI, 16], F32, tag="rw1t")
    nc.scalar.dma_start(rw1t, rw1t_d)
    rw2t = cpool.tile([16, CI], F32, tag="rw2t")
    nc.scalar.dma_start(rw2t, rw2t_d)
    rw3t = cpool.tile([CI, 16], F32, tag="rw3t")
    nc.scalar.dma_start(rw3t, rw3t_d)
    caw1t = cpool.tile([CO, 16], F32, tag="caw1t")
    nc.scalar.dma_start(caw1t, caw1t_d)
    caw2t = cpool.tile([16, CO], F32, tag="caw2t")
    nc.scalar.dma_start(caw2t, caw2t_d)
    gs1 = cvec("gs1", gs1_d, 16)
    bb1 = cvec("bb1", bb1_d, 16)
    gs2 = cvec("gs2", gs2_d, CI)
    bb2 = cvec("bb2", bb2_d, CI)
    gsca1 = cvec("gsca1", gsca1_d, 16)
    bbca1 = cvec("bbca1", bbca1_d, 16)
    gsca2 = cvec("gsca2", gsca2_d, CO)
    bbca2 = cvec("bbca2", bbca2_d, CO)
    gssa = cvec("gssa", gssa_d, 128)
    bssa = cvec("bssa", bssa_d, 128)
    rb3r = cpool.tile([1, E], F32, tag="rb3r")
    nc.scalar.dma_start(rb3r, rb3_d.unsqueeze(0))
    bmask = cpool.tile([128, 8], BF16, tag="bmask")
    nc.scalar.dma_start(bmask, bmask_d)
    msum = cpool.tile([128, 7, 128], BF16, tag="msum")
    nc.scalar.dma_start(msum, msum_d)
    mmax = cpool.tile([128, 7, 128], BF16, tag="mmax")
    nc.scalar.dma_start(mmax, mmax_d)
    identb = cpool.tile([128, 128], BF16, tag="identb")
    nc.scalar.dma_start(identb, identb_d)

    # ---------- routing (both samples) ----------
    rwcols = []
    for b in range(BL):
        psA = spool.tile([128, 1], F32, tag="psA")
        nc.vector.tensor_reduce(psA, xp[b][:, 0:64, :], AX.XY, ALU.add)
        pparts = spool.tile([128, 14], F32, tag="pparts")
        for i in range(13):
            r0 = 64 + 5 * i
            rn = min(5, H + 2 - r0)
            pscr = fpool.tile([128, 5, W + 2], BF16, tag="pscr")
            nc.scalar.activation(
                pscr[:, 0:rn, :], xp[b][:, r0:r0 + rn, :], ACTF.Copy,
                accum_out=pparts[:, i:i + 1])
        nc.vector.memset(pparts[:, 13:14], 0.0)
        psB = spool.tile([128, 1], F32, tag="psB")
        nc.vector.tensor_reduce(psB, pparts, AX.X, ALU.add)
        psum_t = spool.tile([128, 1], F32, tag="psum_t")
        nc.vector.tensor_add(psum_t, psA, psB)

        mm1 = pm.tile([16, 1], F32, tag="m")
        nc.tensor.matmul(mm1, rw1t, psum_t, start=True, stop=True)
        h1 = spool.tile([16, 1], F32, tag="h1")
        nc.scalar.activation(h1, mm1, ACTF.Relu, bias=bb1, scale=gs1)
        mm2 = pm.tile([128, 1], F32, tag="m")
        nc.tensor.matmul(mm2, rw2t, h1, start=True, stop=True)
        gg = spool.tile([128, 1], F32, tag="gg")
        nc.scalar.activation(gg, mm2, ACTF.Sigmoid, bias=bb2, scale=gs2)
        mm3 = pm.tile([1, E], F32, tag="m")
        nc.tensor.matmul(mm3, gg, rw3t, start=True, stop=True)
        lg = spool.tile([1, E], F32, tag="lg")
        nc.vector.tensor_add(lg, mm3, rb3r)
        mx = spool.tile([1, 1], F32, tag="mx")
        nc.vector.tensor_reduce(mx, lg, AX.X, ALU.max)
        mxn = spool.tile([1, 1], F32, tag="mxn")
        nc.vector.tensor_scalar_mul(mxn, mx, -1.0)
        e16 = spool.tile([1, E], F32, tag="e16")
        nc.scalar.activation(e16, lg, ACTF.Exp, bias=mxn, scale=1.0)
        s1 = spool.tile([1, 1], F32, tag="s1")
        nc.vector.tensor_reduce(s1, e16, AX.X, ALU.add)
        rinv = spool.tile([1, 1], F32, tag="rinv")
        nc.vector.reciprocal(rinv, s1)
        rwrow = spool.tile([1, E], F32, tag="rwrow")
        nc.vector.tensor_scalar_mul(rwrow, e16, rinv)
        nc.sync.dma_start(srw_d[b].unsqueeze(0), rwrow)
        rwcol = spool.tile([128, 1], F32, tag=f"rwcol{b}", name=f"rwcol{b}")
        nc.sync.dma_start(rwcol, srw_d[b].unsqueeze(0).broadcast_to([8, E]))
        rwcols.append(rwcol)

    rwblk = spool.tile([128, 2 * RR], BF16, tag="rwblk")
    nc.vector.tensor_scalar_mul(rwblk[:, 0:8], bmask, rwcols[0])
    nc.vector.tensor_scalar_mul(rwblk[:, 8:16], bmask, rwcols[1])

    # ---------- wgen (both samples) ----------
    wsb = [wpool.tile([128, 9, 128], BF16, tag=f"wsb{b}", name=f"wsb{b}")
           for b in range(BL)]
    for og in range(16):
        ec = epool.tile([128, 9, 128], BF16, tag="ec")
        nc.sync.dma_start(ec, ew_d[og])
        pw = pm.tile([128, 9, 16], F32, tag="m")
        for k in range(9):
            nc.tensor.matmul(pw[:, k, :], ec[:, k, :], rwblk,
                             start=True, stop=True)
        if og % 2 == 0:
            nc.scalar.activation(wsb[0][:, :, og * 8:og * 8 + 8],
                                 pw[:, :, 0:8], ACTF.Copy)
            nc.vector.tensor_copy(wsb[1][:, :, og * 8:og * 8 + 8],
                                  pw[:, :, 8:16])
        else:
            nc.vector.tensor_copy(wsb[0][:, :, og * 8:og * 8 + 8],
                                  pw[:, :, 0:8])
            nc.scalar.activation(wsb[1][:, :, og * 8:og * 8 + 8],
                                 pw[:, :, 8:16], ACTF.Copy)

    # CBAM sp-map tiles: pads written once, reused across samples
    spsum_t = spool.tile([128, 134], BF16, tag="spsum_t")   # [h, w+pad]
    spmax_wh = spool.tile([128, 134], BF16, tag="spmax_wh")  # [w, h+pad]
    for t in (spsum_t, spmax_wh):
        nc.vector.memset(t[:, 0:3], 0.0)
        nc.vector.memset(t[:, 131:134], 0.0)

    # ---------- per-sample ----------
    for b in range(BL):
        osb = opool.tile([128, H, W], BF16, tag="osb")
        cparts = spool.tile([128, 32], F32, tag="cparts")

        # conv: 16 strips of 8 rows, PSUM double-buffered
        for sup in range(16):
            pcs = [pc.tile([128, 4, W], F32, tag="c", name=f"pc{b}_{sup}_{g}")
                   for g in range(2)]
            for k in range(9):
                kh, kw = divmod(k, 3)
                lhs = wsb[b][:, k, :]
                for g in range(2):
                    r0 = sup * 8 + g * 4 + kh
                    nc.tensor.matmul(pcs[g], lhs, xp[b][:, r0:r0 + 4, kw:kw + W],
                                     start=(k == 0), stop=(k == 8))
            for g in range(2):
                hr = sup * 8 + g * 4
                nc.scalar.activation(
                    osb[:, hr:hr + 4, :], pcs[g], ACTF.Copy,
                    accum_out=cparts[:, sup * 2 + g:sup * 2 + g + 1])

        # SE MLP
        cps = spool.tile([128, 1], F32, tag="cps")
        nc.vector.tensor_reduce(cps, cparts, AX.X, ALU.add)
        se1 = pm.tile([16, 1], F32, tag="m")
        nc.tensor.matmul(se1, caw1t, cps, start=True, stop=True)
        ch = spool.tile([16, 1], F32, tag="ch")
        nc.scalar.activation(ch, se1, ACTF.Relu, bias=bbca1, scale=gsca1)
        se2 = pm.tile([128, 1], F32, tag="m")
        nc.tensor.matmul(se2, caw2t, ch, start=True, stop=True)
        cw = spool.tile([128, 1], F32, tag="cw")
        nc.scalar.activation(cw, se2, ACTF.Sigmoid, bias=bbca2, scale=gsca2)
        cwb = spool.tile([128, 1], BF16, tag="cwb")
        nc.vector.tensor_copy(cwb, cw)
        diagcw = spool.tile([128, 128], BF16, tag="diagcw")
        nc.vector.tensor_scalar_mul(diagcw, identb, cw)

        # CBAM stats: per 4-row group, 4 scaled-transpose matmuls (max)
        # + 1 cw-weighted column matmul (mean)
        for c in range(4):
            mf = fpool.tile([1, 4096], BF16, tag="mf")
            for j in range(8):
                h0 = c * 32 + j * 4
                ptt = pc.tile([128, 4, 128], F32, tag="c", name=f"ptt{b}_{c}_{j}")
                for i in range(4):
                    nc.tensor.matmul(ptt[:, i, :], osb[:, h0 + i, :], diagcw,
                                     start=True, stop=True)
                pmean = pm.tile([1, 512], F32, tag="m")
                nc.tensor.matmul(pmean, cwb, osb[:, h0:h0 + 4, :],
                                 start=True, stop=True)
                nc.vector.tensor_reduce(spmax_wh[:, 3 + h0:3 + h0 + 4], ptt,
                                        AX.X, ALU.max)
                if j % 2 == 0:
                    nc.scalar.activation(mf[:, j * 512:(j + 1) * 512], pmean,
                                         ACTF.Copy)
                else:
                    nc.vector.tensor_copy(mf[:, j * 512:(j + 1) * 512], pmean)
            nc.sync.dma_start(spsum_t[c * 32:(c + 1) * 32, 3:131], mf)

        # CBAM 7x7 conv: banded matmuls in both layouts, folded via identity
        pswW = pm.tile([128, 128], F32, tag="m")
        for t in range(7):
            nc.tensor.matmul(pswW, mmax[:, t, :], spmax_wh[:, t:t + 128],
                             start=(t == 0), stop=(t == 6))
        swW = spool.tile([128, 128], BF16, tag="swW")
        nc.scalar.activation(swW, pswW, ACTF.Copy)
        psw = pm.tile([128, 128], F32, tag="m")
        for t in range(7):
            nc.tensor.matmul(psw, msum[:, t, :], spsum_t[:, t:t + 128],
                             start=(t == 0), stop=False)
        nc.tensor.matmul(psw, swW, identb, start=False, stop=True)
        swsb = spool.tile([128, 128], BF16, tag="swsb")
        nc.scalar.activation(swsb, psw, ACTF.Sigmoid, bias=bssa, scale=gssa)
        nc.sync.dma_start(ssw_d[b].rearrange("(h w) -> h w", h=128), swsb)

        # final: out = (osb*sw)*cw + x, in place, chunked
        for q in range(8):
            swbc = fpool.tile([128, 16, 128], BF16, tag="swbc")
            nc.sync.dma_start(
                swbc,
                ssw_d[b, q * 2048:(q + 1) * 2048].unsqueeze(0)
                .partition_broadcast(128))
            sl = osb[:, q * 16:(q + 1) * 16, :]
            nc.vector.tensor_mul(sl, sl, swbc)
            nc.vector.scalar_tensor_tensor(
                sl, sl, cw, xp[b][:, 1 + q * 16:17 + q * 16, 1:W + 1],
                ALU.mult, ALU.add)
            nc.sync.dma_start(out_d[b, :, q * 16:(q + 1) * 16, :], sl)


def _host_prep(inp):
    import ml_dtypes
    experts = np.ascontiguousarray(inp["experts"], dtype=np.float32)
    ew = experts.reshape(E, 16, 8, CI, 9)          # [e, og, o', i, k]
    ew = ew.transpose(1, 2, 0, 4, 3)               # [og, o', e, k, i]
    ew = np.ascontiguousarray(ew).reshape(16, 128, 9, 128)

    bm = np.zeros((8, 16, 8), dtype=ml_dtypes.bfloat16)
    for j in range(8):
        bm[j, :, j] = 1.0
    bm = bm.reshape(128, 8)

    # banded Toeplitz matrices for the 7x7 CBAM conv
    saw = np.asarray(inp["sa_w"], np.float32).reshape(2, 7, 7)
    # sum map in [h, w]: contract over h, shift over w -> index dw
    Ms = np.zeros((7, 128, 128), dtype=np.float32)
    # max map in [w, h]: contract over w, shift over h -> index dh
    Mm = np.zeros((7, 128, 128), dtype=np.float32)
    hp = np.arange(128)
    for dh in range(7):
        for dw in range(7):
            src = hp + dh - 3
            v = (src >= 0) & (src < 128)
            Ms[dw, src[v], hp[v]] += saw[0, dh, dw] / CO
            src2 = hp + dw - 3
            v2 = (src2 >= 0) & (src2 < 128)
            Mm[dh, src2[v2], hp[v2]] += saw[1, dh, dw]
    msum = np.ascontiguousarray(Ms.transpose(1, 0, 2))  # [h, dw, h']
    mmax = np.ascontiguousarray(Mm.transpose(1, 0, 2))  # [w, dh, w']

    x = np.asarray(inp["x"], np.float32)
    xpad = np.zeros((B, CI, H + 2, W + 2), dtype=ml_dtypes.bfloat16)
    xpad[:, :, 1:H + 1, 1:W + 1] = x

    shared = {
        "experts_w": ew.astype(ml_dtypes.bfloat16),
        "rw1t": np.ascontiguousarray(inp["rw1"].T, dtype=np.float32),
        "rw2t": np.ascontiguousarray(inp["rw2"].T, dtype=np.float32),
        "rw3t": np.ascontiguousarray(inp["rw3"].T, dtype=np.float32),
        "caw1t": np.ascontiguousarray(inp["ca_w1"].T, dtype=np.float32),
        "caw2t": np.ascontiguousarray(inp["ca_w2"].T, dtype=np.float32),
        "gs1": np.asarray(inp["rbn1_g"], np.float32) * (BNS / HW),
        "bb1": np.asarray(inp["rbn1_b"], np.float32),
        "gs2": np.asarray(inp["rbn2_g"], np.float32) * BNS,
        "bb2": np.asarray(inp["rbn2_b"], np.float32),
        "rb3": np.asarray(inp["rb3"], np.float32),
        "gsca1": np.asarray(inp["ca_bn1_g"], np.float32) * (BNS / HW),
        "bbca1": np.asarray(inp["ca_bn1_b"], np.float32),
        "gsca2": np.asarray(inp["ca_bn2_g"], np.float32) * BNS,
        "bbca2": np.asarray(inp["ca_bn2_b"], np.float32),
        "msum": msum.astype(ml_dtypes.bfloat16),
        "mmax": mmax.astype(ml_dtypes.bfloat16),
        "identb": np.eye(128, dtype=ml_dtypes.bfloat16),
        "gssa": np.full(128, float(inp["sa_bn_g"][0]) * BNS, np.float32),
        "bssa": np.full(128, float(inp["sa_bn_b"][0]), np.float32),
        "bmask": bm,
    }
    in_maps = []
    for c in range(NCORES):
        m = dict(shared)
        m["xpad"] = np.ascontiguousarray(xpad[BL * c:BL * (c + 1)])
        in_maps.append(m)
    return in_maps


def get_module():
    if "nc" not in _CACHE:
        _CACHE["nc"] = _build_module()
    return _CACHE["nc"]


def kernel(**inputs):
    nc = get_module()
    in_maps = _host_prep(inputs)
    res = run_bass_kernel_spmd(nc, in_maps, core_ids=list(range(NCORES)))
    out = np.concatenate([r["out"] for r in res.results], axis=0)
    return out.astype(np.float32)


# revision 5
# speedup vs baseline: 1.7745x; 1.0366x over previous
"""Trainium2 Bass kernel for EnhancedCondConv2d (moe_routing).

Data-parallel over batch: 8 cores x 2 samples each. Full inputs in,
full outputs back.

Per-core program:
  1. routing (both samples): avgpool(x) -> tiny MLP -> softmax
  2. wgen (both samples fused): w[b] = sum_e rw[e]*experts[e] as 144
     block-diag matmuls with contiguous (FWL-friendly) expert layout
  3. conv per sample: 9 PSUM-accumulated shifted matmuls per 8-row
     strip, double-buffered PSUM so PE never stalls on eviction
  4. SE channel attention folded into PSUM eviction accumulators
  5. CBAM stats: cw-scaled transpose of each h-plane via one matmul
     against diag(cw) -> DVE max-reduce gives the channel max in
     [w-part, h] layout; channel mean via cw-weighted column matmul in
     flat [h, w] layout. 7x7 conv as banded-Toeplitz matmuls in both
     layouts, folded into one PSUM via an identity matmul.
  6. final: out = (osb*sw)*cw + x in place, all bf16 (host casts f32)
"""

import math
from contextlib import ExitStack

import numpy as np

import concourse.bass as bass
import concourse.bacc as bacc
import concourse.mybir as mybir
import concourse.tile as tile
from concourse.bass_utils import run_bass_kernel_spmd

F32 = mybir.dt.float32
BF16 = mybir.dt.bfloat16
AX = mybir.AxisListType
ALU = mybir.AluOpType
ACTF = mybir.ActivationFunctionType

B, CI, CO, H, W, E, RR = 16, 128, 128, 128, 128, 16, 8
NCORES = 8
BL = B // NCORES  # 2 samples per core
EPS = 1e-5
HW = H * W
BNS = 1.0 / math.sqrt(1.0 + EPS)

_CACHE = {}


def _build_module():
    nc = bacc.Bacc("TRN2", target_bir_lowering=False, debug=False)

    xp_d = nc.dram_tensor("xpad", [BL, CI, H + 2, W + 2], BF16,
                          kind="ExternalInput").ap()
    ew_d = nc.dram_tensor("experts_w", [16, 128, 9, 128], BF16,
                          kind="ExternalInput").ap()
    rw1t_d = nc.dram_tensor("rw1t", [CI, 16], F32, kind="ExternalInput").ap()
    rw2t_d = nc.dram_tensor("rw2t", [16, CI], F32, kind="ExternalInput").ap()
    rw3t_d = nc.dram_tensor("rw3t", [CI, 16], F32, kind="ExternalInput").ap()
    caw1t_d = nc.dram_tensor("caw1t", [CO, 16], F32, kind="ExternalInput").ap()
    caw2t_d = nc.dram_tensor("caw2t", [16, CO], F32, kind="ExternalInput").ap()
    gs1_d = nc.dram_tensor("gs1", [16], F32, kind="ExternalInput").ap()
    bb1_d = nc.dram_tensor("bb1", [16], F32, kind="ExternalInput").ap()
    gs2_d = nc.dram_tensor("gs2", [CI], F32, kind="ExternalInput").ap()
    bb2_d = nc.dram_tensor("bb2", [CI], F32, kind="ExternalInput").ap()
    rb3_d = nc.dram_tensor("rb3", [E], F32, kind="ExternalInput").ap()
    gsca1_d = nc.dram_tensor("gsca1", [16], F32, kind="ExternalInput").ap()
    bbca1_d = nc.dram_tensor("bbca1", [16], F32, kind="ExternalInput").ap()
    gsca2_d = nc.dram_tensor("gsca2", [CO], F32, kind="ExternalInput").ap()
    bbca2_d = nc.dram_tensor("bbca2", [CO], F32, kind="ExternalInput").ap()
    msum_d = nc.dram_tensor("msum", [128, 7, 128], BF16,
                            kind="ExternalInput").ap()
    mmax_d = nc.dram_tensor("mmax", [128, 7, 128], BF16,
                            kind="ExternalInput").ap()
    identb_d = nc.dram_tensor("identb", [128, 128], BF16,
                              kind="ExternalInput").ap()
    gssa_d = nc.dram_tensor("gssa", [128], F32, kind="ExternalInput").ap()
    bssa_d = nc.dram_tensor("bssa", [128], F32, kind="ExternalInput").ap()
    bmask_d = nc.dram_tensor("bmask", [128, 8], BF16, kind="ExternalInput").ap()

    out_d = nc.dram_tensor("out", [BL, CO, H, W], BF16,
                           kind="ExternalOutput").ap()

    srw_d = nc.dram_tensor("scr_rw", [BL, E], F32).ap()
    ssw_d = nc.dram_tensor("scr_sw", [BL, HW], BF16).ap()

    with tile.TileContext(nc) as tc, ExitStack() as ctx:
        _kernel_body(ctx, tc, xp_d, ew_d, rw1t_d, rw2t_d, rw3t_d,
                     caw1t_d, caw2t_d, gs1_d, bb1_d, gs2_d, bb2_d, rb3_d,
                     gsca1_d, bbca1_d, gsca2_d, bbca2_d, msum_d, mmax_d,
                     identb_d, gssa_d, bssa_d, bmask_d, out_d, srw_d, ssw_d)
    nc.compile()
    return nc


def _kernel_body(ctx, tc, xp_d, ew_d, rw1t_d, rw2t_d, rw3t_d,
                 caw1t_d, caw2t_d, gs1_d, bb1_d, gs2_d, bb2_d, rb3_d,
                 gsca1_d, bbca1_d, gsca2_d, bbca2_d, msum_d, mmax_d,
                 identb_d, gssa_d, bssa_d, bmask_d, out_d, srw_d, ssw_d):
    nc = tc.nc

    cpool = ctx.enter_context(tc.tile_pool(name="const", bufs=1))
    xpool = ctx.enter_context(tc.tile_pool(name="xp", bufs=1))
    opool = ctx.enter_context(tc.tile_pool(name="ob", bufs=2))
    wpool = ctx.enter_context(tc.tile_pool(name="wp", bufs=1))
    epool = ctx.enter_context(tc.tile_pool(name="ep", bufs=4))
    spool = ctx.enter_context(tc.tile_pool(name="sp", bufs=2))
    fpool = ctx.enter_context(tc.tile_pool(name="fp", bufs=2))

    pc = ctx.enter_context(tc.tile_pool(name="pc", bufs=4, space="PSUM"))
    pm = ctx.enter_context(tc.tile_pool(name="pm", bufs=4, space="PSUM"))

    # ---------- input loads first (engines are DMA-starved at head) ----------
    xp = []
    for b in range(BL):
        t = xpool.tile([128, H + 2, W + 2], BF16, tag=f"xp{b}", name=f"xp{b}")
        nc.sync.dma_start(t[:, 0:66, :], xp_d[b, :, 0:66, :])
        nc.sync.dma_start(t[:, 66:130, :], xp_d[b, :, 66:130, :])
        xp.append(t)

    # ---------- constants (issued on the scalar HWDGE queue) ----------
    def cvec(tag, src, n):
        t = cpool.tile([n, 1], F32, tag=tag, name=tag)
        nc.scalar.dma_start(t, src.unsqueeze(1))
        return t

    rw1t = cpool.tile([CI, 16], F32, tag="rw1t")
    nc.scalar.dma_start(rw1t, rw1t_d)
    rw2t = cpool.tile([16, CI], F32, tag="rw2t")
    nc.scalar.dma_start(rw2t, rw2t_d)
    rw3t = cpool.tile([CI, 16], F32, tag="rw3t")
    nc.scalar.dma_start(rw3t, rw3t_d)
    caw1t = cpool.tile([CO, 16], F32, tag="caw1t")
    nc.scalar.dma_start(caw1t, caw1t_d)
    caw2t = cpool.tile([16, CO], F32, tag="caw2t")
    nc.scalar.dma_start(caw2t, caw2t_d)
    gs1 = cvec("gs1", gs1_d, 16)
    bb1 = cvec("bb1", bb1_d, 16)
    gs2 = cvec("gs2", gs2_d, CI)
    bb2 = cvec("bb2", bb2_d, CI)
    gsca1 = cvec("gsca1", gsca1_d, 16)
    bbca1 = cvec("bbca1", bbca1_d, 16)
    gsca2 = cvec("gsca2", gsca2_d, CO)
    bbca2 = cvec("bbca2", bbca2_d, CO)
    gssa = cvec("gssa", gssa_d, 128)
    bssa = cvec("bssa", bssa_d, 128)
    rb3r = cpool.tile([1, E], F32, tag="rb3r")
    nc.scalar.dma_start(rb3r, rb3_d.unsqueeze(0))
    bmask = cpool.tile([128, 8], BF16, tag="bmask")
    nc.scalar.dma_start(bmask, bmask_d)
    msum = cpool.tile([128, 7, 128], BF16, tag="msum")
    nc.scalar.dma_start(msum, msum_d)
    mmax = cpool.tile([128, 7, 128], BF16, tag="mmax")
    nc.scalar.dma_start(mmax, mmax_d)
    identb = cpool.tile([128, 128], BF16, tag="identb")
    nc.scalar.dma_start(identb, identb_d)

    # expert chunk prefetch (DMA only; consumed by wgen below)
    ecs = []
    for og in range(16):
        ec = epool.tile([128, 9, 128], BF16, tag="ec", name=f"ec{og}")
        nc.sync.dma_start(ec, ew_d[og])
        ecs.append(ec)

    # ---------- routing (both samples) ----------
    rwcols = []
    for b in range(BL):
        psA = spool.tile([128, 1], F32, tag="psA")
        nc.vector.tensor_reduce(psA, xp[b][:, 0:64, :], AX.XY, ALU.add)
        pparts = spool.tile([128, 14], F32, tag="pparts")
        for i in range(13):
            r0 = 64 + 5 * i
            rn = min(5, H + 2 - r0)
            pscr = fpool.tile([128, 5, W + 2], BF16, tag="pscr")
            nc.scalar.activation(
                pscr[:, 0:rn, :], xp[b][:, r0:r0 + rn, :], ACTF.Copy,
                accum_out=pparts[:, i:i + 1])
        nc.vector.memset(pparts[:, 13:14], 0.0)
        psB = spool.tile([128, 1], F32, tag="psB")
        nc.vector.tensor_reduce(psB, pparts, AX.X, ALU.add)
        psum_t = spool.tile([128, 1], F32, tag="psum_t")
        nc.vector.tensor_add(psum_t, psA, psB)

        mm1 = pm.tile([16, 1], F32, tag="m")
        nc.tensor.matmul(mm1, rw1t, psum_t, start=True, stop=True)
        h1 = spool.tile([16, 1], F32, tag="h1")
        nc.scalar.activation(h1, mm1, ACTF.Relu, bias=bb1, scale=gs1)
        mm2 = pm.tile([128, 1], F32, tag="m")
        nc.tensor.matmul(mm2, rw2t, h1, start=True, stop=True)
        gg = spool.tile([128, 1], F32, tag="gg")
        nc.scalar.activation(gg, mm2, ACTF.Sigmoid, bias=bb2, scale=gs2)
        mm3 = pm.tile([1, E], F32, tag="m")
        nc.tensor.matmul(mm3, gg, rw3t, start=True, stop=True)
        lg = spool.tile([1, E], F32, tag="lg")
        nc.vector.tensor_add(lg, mm3, rb3r)
        mx = spool.tile([1, 1], F32, tag="mx")
        nc.vector.tensor_reduce(mx, lg, AX.X, ALU.max)
        mxn = spool.tile([1, 1], F32, tag="mxn")
        nc.vector.tensor_scalar_mul(mxn, mx, -1.0)
        e16 = spool.tile([1, E], F32, tag="e16")
        nc.scalar.activation(e16, lg, ACTF.Exp, bias=mxn, scale=1.0)
        s1 = spool.tile([1, 1], F32, tag="s1")
        nc.vector.tensor_reduce(s1, e16, AX.X, ALU.add)
        rinv = spool.tile([1, 1], F32, tag="rinv")
        nc.vector.reciprocal(rinv, s1)
        rwrow = spool.tile([1, E], F32, tag="rwrow")
        nc.vector.tensor_scalar_mul(rwrow, e16, rinv)
        nc.sync.dma_start(srw_d[b].unsqueeze(0), rwrow)
        rwcol = spool.tile([128, 1], F32, tag=f"rwcol{b}", name=f"rwcol{b}")
        nc.sync.dma_start(rwcol, srw_d[b].unsqueeze(0).broadcast_to([8, E]))
        rwcols.append(rwcol)

    rwblk = spool.tile([128, 2 * RR], BF16, tag="rwblk")
    nc.vector.tensor_scalar_mul(rwblk[:, 0:8], bmask, rwcols[0])
    nc.vector.tensor_scalar_mul(rwblk[:, 8:16], bmask, rwcols[1])

    # ---------- wgen (both samples) ----------
    wsb = [wpool.tile([128, 9, 128], BF16, tag=f"wsb{b}", name=f"wsb{b}")
           for b in range(BL)]
    for og in range(16):
        ec = ecs[og]
        pw = pm.tile([128, 9, 16], F32, tag="m")
        for k in range(9):
            nc.tensor.matmul(pw[:, k, :], ec[:, k, :], rwblk,
                             start=True, stop=True)
        if og % 2 == 0:
            nc.scalar.activation(wsb[0][:, :, og * 8:og * 8 + 8],
                                 pw[:, :, 0:8], ACTF.Copy)
            nc.vector.tensor_copy(wsb[1][:, :, og * 8:og * 8 + 8],
                                  pw[:, :, 8:16])
        else:
            nc.vector.tensor_copy(wsb[0][:, :, og * 8:og * 8 + 8],
                                  pw[:, :, 0:8])
            nc.scalar.activation(wsb[1][:, :, og * 8:og * 8 + 8],
                                 pw[:, :, 8:16], ACTF.Copy)

    # CBAM sp-map tiles: pads written once, reused across samples
    spsum_t = spool.tile([128, 134], BF16, tag="spsum_t")   # [h, w+pad]
    spmax_wh = spool.tile([128, 134], BF16, tag="spmax_wh")  # [w, h+pad]
    for t in (spsum_t, spmax_wh):
        nc.vector.memset(t[:, 0:3], 0.0)
        nc.vector.memset(t[:, 131:134], 0.0)

    # ---------- per-sample ----------
    for b in range(BL):
        osb = opool.tile([128, H, W], BF16, tag="osb")
        cparts = spool.tile([128, 32], F32, tag="cparts")

        # conv: 16 strips of 8 rows, PSUM double-buffered
        for sup in range(16):
            pcs = [pc.tile([128, 4, W], F32, tag="c", name=f"pc{b}_{sup}_{g}")
                   for g in range(2)]
            for k in range(9):
                kh, kw = divmod(k, 3)
                lhs = wsb[b][:, k, :]
                for g in range(2):
                    r0 = sup * 8 + g * 4 + kh
                    nc.tensor.matmul(pcs[g], lhs, xp[b][:, r0:r0 + 4, kw:kw + W],
                                     start=(k == 0), stop=(k == 8))
            for g in range(2):
                hr = sup * 8 + g * 4
                nc.scalar.activation(
                    osb[:, hr:hr + 4, :], pcs[g], ACTF.Copy,
                    accum_out=cparts[:, sup * 2 + g:sup * 2 + g + 1])

        # SE MLP
        cps = spool.tile([128, 1], F32, tag="cps")
        nc.vector.tensor_reduce(cps, cparts, AX.X, ALU.add)
        se1 = pm.tile([16, 1], F32, tag="m")
        nc.tensor.matmul(se1, caw1t, cps, start=True, stop=True)
        ch = spool.tile([16, 1], F32, tag="ch")
        nc.scalar.activation(ch, se1, ACTF.Relu, bias=bbca1, scale=gsca1)
        se2 = pm.tile([128, 1], F32, tag="m")
        nc.tensor.matmul(se2, caw2t, ch, start=True, stop=True)
        cw = spool.tile([128, 1], F32, tag="cw")
        nc.scalar.activation(cw, se2, ACTF.Sigmoid, bias=bbca2, scale=gsca2)
        cwb = spool.tile([128, 1], BF16, tag="cwb")
        nc.vector.tensor_copy(cwb, cw)
        diagcw = spool.tile([128, 128], BF16, tag="diagcw")
        nc.vector.tensor_scalar_mul(diagcw, identb, cw)

        # CBAM stats: per 4-row group, 4 scaled-transpose matmuls (max)
        # + 1 cw-weighted column matmul (mean)
        for c in range(4):
            mf = fpool.tile([1, 4096], BF16, tag="mf")
            for j in range(8):
                h0 = c * 32 + j * 4
                ptt = pc.tile([128, 4, 128], F32, tag="c", name=f"ptt{b}_{c}_{j}")
                for i in range(4):
                    nc.tensor.matmul(ptt[:, i, :], osb[:, h0 + i, :], diagcw,
                                     start=True, stop=True)
                pmean = pm.tile([1, 512], F32, tag="m")
                nc.tensor.matmul(pmean, cwb, osb[:, h0:h0 + 4, :],
                                 start=True, stop=True)
                nc.vector.tensor_reduce(spmax_wh[:, 3 + h0:3 + h0 + 4], ptt,
                                        AX.X, ALU.max)
                nc.scalar.activation(mf[:, j * 512:(j + 1) * 512], pmean,
                                     ACTF.Copy)
            nc.sync.dma_start(spsum_t[c * 32:(c + 1) * 32, 3:131], mf)

        # CBAM 7x7 conv: banded matmuls in both layouts, folded via identity
        pswW = pm.tile([128, 128], F32, tag="m")
        for t in range(7):
            nc.tensor.matmul(pswW, mmax[:, t, :], spmax_wh[:, t:t + 128],
                             start=(t == 0), stop=(t == 6))
        swW = spool.tile([128, 128], BF16, tag="swW")
        nc.scalar.activation(swW, pswW, ACTF.Copy)
        psw = pm.tile([128, 128], F32, tag="m")
        for t in range(7):
            nc.tensor.matmul(psw, msum[:, t, :], spsum_t[:, t:t + 128],
                             start=(t == 0), stop=False)
        nc.tensor.matmul(psw, swW, identb, start=False, stop=True)
        swsb = spool.tile([128, 128], BF16, tag="swsb")
        nc.scalar.activation(swsb, psw, ACTF.Sigmoid, bias=bssa, scale=gssa)
        nc.sync.dma_start(ssw_d[b].rearrange("(h w) -> h w", h=128), swsb)

        # final: out = (osb*sw)*cw + x, in place, chunked
        for q in range(8):
            swbc = fpool.tile([128, 16, 128], BF16, tag="swbc")
            nc.sync.dma_start(
                swbc,
                ssw_d[b, q * 2048:(q + 1) * 2048].unsqueeze(0)
                .partition_broadcast(128))
            sl = osb[:, q * 16:(q + 1) * 16, :]
            nc.vector.tensor_mul(sl, sl, swbc)
            nc.vector.scalar_tensor_tensor(
                sl, sl, cw, xp[b][:, 1 + q * 16:17 + q * 16, 1:W + 1],
                ALU.mult, ALU.add)
            nc.scalar.dma_start(out_d[b, :, q * 16:(q + 1) * 16, :], sl)


def _host_prep(inp):
    import ml_dtypes
    experts = np.ascontiguousarray(inp["experts"], dtype=np.float32)
    ew = experts.reshape(E, 16, 8, CI, 9)          # [e, og, o', i, k]
    ew = ew.transpose(1, 2, 0, 4, 3)               # [og, o', e, k, i]
    ew = np.ascontiguousarray(ew).reshape(16, 128, 9, 128)

    bm = np.zeros((8, 16, 8), dtype=ml_dtypes.bfloat16)
    for j in range(8):
        bm[j, :, j] = 1.0
    bm = bm.reshape(128, 8)

    # banded Toeplitz matrices for the 7x7 CBAM conv
    saw = np.asarray(inp["sa_w"], np.float32).reshape(2, 7, 7)
    # sum map in [h, w]: contract over h, shift over w -> index dw
    Ms = np.zeros((7, 128, 128), dtype=np.float32)
    # max map in [w, h]: contract over w, shift over h -> index dh
    Mm = np.zeros((7, 128, 128), dtype=np.float32)
    hp = np.arange(128)
    for dh in range(7):
        for dw in range(7):
            src = hp + dh - 3
            v = (src >= 0) & (src < 128)
            Ms[dw, src[v], hp[v]] += saw[0, dh, dw] / CO
            src2 = hp + dw - 3
            v2 = (src2 >= 0) & (src2 < 128)
            Mm[dh, src2[v2], hp[v2]] += saw[1, dh, dw]
    msum = np.ascontiguousarray(Ms.transpose(1, 0, 2))  # [h, dw, h']
    mmax = np.ascontiguousarray(Mm.transpose(1, 0, 2))  # [w, dh, w']

    x = np.asarray(inp["x"], np.float32)
    xpad = np.zeros((B, CI, H + 2, W + 2), dtype=ml_dtypes.bfloat16)
    xpad[:, :, 1:H + 1, 1:W + 1] = x

    shared = {
        "experts_w": ew.astype(ml_dtypes.bfloat16),
        "rw1t": np.ascontiguousarray(inp["rw1"].T, dtype=np.float32),
        "rw2t": np.ascontiguousarray(inp["rw2"].T, dtype=np.float32),
        "rw3t": np.ascontiguousarray(inp["rw3"].T, dtype=np.float32),
        "caw1t": np.ascontiguousarray(inp["ca_w1"].T, dtype=np.float32),
        "caw2t": np.ascontiguousarray(inp["ca_w2"].T, dtype=np.float32),
        "gs1": np.asarray(inp["rbn1_g"], np.float32) * (BNS / HW),
        "bb1": np.asarray(inp["rbn1_b"], np.float32),
        "gs2": np.asarray(inp["rbn2_g"], np.float32) * BNS,
        "bb2": np.asarray(inp["rbn2_b"], np.float32),
        "rb3": np.asarray(inp["rb3"], np.float32),
        "gsca1": np.asarray(inp["ca_bn1_g"], np.float32) * (BNS / HW),
        "bbca1": np.asarray(inp["ca_bn1_b"], np.float32),
        "gsca2": np.asarray(inp["ca_bn2_g"], np.float32) * BNS,
        "bbca2": np.asarray(inp["ca_bn2_b"], np.float32),
        "msum": msum.astype(ml_dtypes.bfloat16),
        "mmax": mmax.astype(ml_dtypes.bfloat16),
        "identb": np.eye(128, dtype=ml_dtypes.bfloat16),
        "gssa": np.full(128, float(inp["sa_bn_g"][0]) * BNS, np.float32),
        "bssa": np.full(128, float(inp["sa_bn_b"][0]), np.float32),
        "bmask": bm,
    }
    in_maps = []
    for c in range(NCORES):
        m = dict(shared)
        m["xpad"] = np.ascontiguousarray(xpad[BL * c:BL * (c + 1)])
        in_maps.append(m)
    return in_maps


def get_module():
    if "nc" not in _CACHE:
        _CACHE["nc"] = _build_module()
    return _CACHE["nc"]


def kernel(**inputs):
    nc = get_module()
    in_maps = _host_prep(inputs)
    res = run_bass_kernel_spmd(nc, in_maps, core_ids=list(range(NCORES)))
    out = np.concatenate([r["out"] for r in res.results], axis=0)
    return out.astype(np.float32)


# revision 6
# speedup vs baseline: 1.8223x; 1.0269x over previous
"""Trainium2 Bass kernel for EnhancedCondConv2d (moe_routing).

Data-parallel over batch: 8 cores x 2 samples each. Full inputs in,
full outputs back.

Per-core program:
  1. routing (both samples): avgpool(x) -> tiny MLP -> softmax
  2. wgen (both samples fused): w[b] = sum_e rw[e]*experts[e] as 144
     block-diag matmuls with contiguous (FWL-friendly) expert layout
  3. conv per sample: 9 PSUM-accumulated shifted matmuls per 8-row
     strip, double-buffered PSUM so PE never stalls on eviction
  4. SE channel attention folded into PSUM eviction accumulators
  5. CBAM stats: cw-scaled transpose of each h-plane via one matmul
     against diag(cw) -> DVE max-reduce gives the channel max in
     [w-part, h] layout; channel mean via cw-weighted column matmul in
     flat [h, w] layout. 7x7 conv as banded-Toeplitz matmuls in both
     layouts, folded into one PSUM via an identity matmul. Sample 0's
     stats interleave into sample 1's conv emission so PE stays dense.
  6. final: out = (osb*sw)*cw + x in place, all bf16 (host casts f32)
"""

import math
from contextlib import ExitStack

import numpy as np

import concourse.bass as bass
import concourse.bacc as bacc
import concourse.mybir as mybir
import concourse.tile as tile
from concourse.bass_utils import run_bass_kernel_spmd

F32 = mybir.dt.float32
BF16 = mybir.dt.bfloat16
AX = mybir.AxisListType
ALU = mybir.AluOpType
ACTF = mybir.ActivationFunctionType

B, CI, CO, H, W, E, RR = 16, 128, 128, 128, 128, 16, 8
NCORES = 8
BL = B // NCORES  # 2 samples per core
EPS = 1e-5
HW = H * W
BNS = 1.0 / math.sqrt(1.0 + EPS)

_CACHE = {}


def _build_module():
    nc = bacc.Bacc("TRN2", target_bir_lowering=False, debug=False)

    xp_d = nc.dram_tensor("xpad", [BL, CI, H + 2, W + 2], BF16,
                          kind="ExternalInput").ap()
    ew_d = nc.dram_tensor("experts_w", [16, 128, 9, 128], BF16,
                          kind="ExternalInput").ap()
    pka_d = nc.dram_tensor("packa", [128, 54], F32, kind="ExternalInput").ap()
    pkb_d = nc.dram_tensor("packb", [16, 260], F32, kind="ExternalInput").ap()
    pkc_d = nc.dram_tensor("packc", [128, 1928], BF16,
                           kind="ExternalInput").ap()
    rb3_d = nc.dram_tensor("rb3", [E], F32, kind="ExternalInput").ap()

    out_d = nc.dram_tensor("out", [BL, CO, H, W], BF16,
                           kind="ExternalOutput").ap()

    srw_d = nc.dram_tensor("scr_rw", [BL, E], F32).ap()
    ssw_d = nc.dram_tensor("scr_sw", [BL, HW], BF16).ap()

    with tile.TileContext(nc) as tc, ExitStack() as ctx:
        _kernel_body(ctx, tc, xp_d, ew_d, pka_d, pkb_d, pkc_d, rb3_d,
                     out_d, srw_d, ssw_d)
    nc.compile()
    return nc


def _kernel_body(ctx, tc, xp_d, ew_d, pka_d, pkb_d, pkc_d, rb3_d,
                 out_d, srw_d, ssw_d):
    nc = tc.nc

    cpool = ctx.enter_context(tc.tile_pool(name="const", bufs=1))
    xpool = ctx.enter_context(tc.tile_pool(name="xp", bufs=1))
    opool = ctx.enter_context(tc.tile_pool(name="ob", bufs=2))
    wpool = ctx.enter_context(tc.tile_pool(name="wp", bufs=1))
    epool = ctx.enter_context(tc.tile_pool(name="ep", bufs=4))
    spool = ctx.enter_context(tc.tile_pool(name="sp", bufs=2))
    fpool = ctx.enter_context(tc.tile_pool(name="fp", bufs=2))

    pc = ctx.enter_context(tc.tile_pool(name="pc", bufs=4, space="PSUM"))
    pm = ctx.enter_context(tc.tile_pool(name="pm", bufs=4, space="PSUM"))

    # ---------- bulk loads in priority order, all on the sync ring ----------
    xp0 = xpool.tile([128, H + 2, W + 2], BF16, tag="xp0")
    nc.sync.dma_start(xp0[:, 0:66, :], xp_d[0, :, 0:66, :])
    nc.sync.dma_start(xp0[:, 66:130, :], xp_d[0, :, 66:130, :])

    pka = cpool.tile([128, 54], F32, tag="pka")
    nc.sync.dma_start(pka, pka_d)
    pkb = cpool.tile([16, 260], F32, tag="pkb")
    nc.sync.dma_start(pkb, pkb_d)
    rb3r = cpool.tile([1, E], F32, tag="rb3r")
    nc.sync.dma_start(rb3r, rb3_d.unsqueeze(0))

    xp1 = xpool.tile([128, H + 2, W + 2], BF16, tag="xp1")
    nc.sync.dma_start(xp1[:, 0:66, :], xp_d[1, :, 0:66, :])
    nc.sync.dma_start(xp1[:, 66:130, :], xp_d[1, :, 66:130, :])
    xp = [xp0, xp1]

    pkc = cpool.tile([128, 1928], BF16, tag="pkc")
    nc.sync.dma_start(pkc, pkc_d)

    ecs = []
    for og in range(16):
        ec = epool.tile([128, 9, 128], BF16, tag="ec", name=f"ec{og}")
        nc.sync.dma_start(ec, ew_d[og])
        ecs.append(ec)

    # const views
    rw1t = pka[:, 0:16]
    rw3t = pka[:, 16:32]
    caw1t = pka[:, 32:48]
    gs2 = pka[:, 48:49]
    bb2 = pka[:, 49:50]
    gsca2 = pka[:, 50:51]
    bbca2 = pka[:, 51:52]
    gssa = pka[:, 52:53]
    bssa = pka[:, 53:54]
    rw2t = pkb[:, 0:128]
    caw2t = pkb[:, 128:256]
    gs1 = pkb[:, 256:257]
    bb1 = pkb[:, 257:258]
    gsca1 = pkb[:, 258:259]
    bbca1 = pkb[:, 259:260]
    msum = pkc[:, 0:896].rearrange("p (t i) -> p t i", t=7)
    mmax = pkc[:, 896:1792].rearrange("p (t i) -> p t i", t=7)
    identb = pkc[:, 1792:1920]
    bmask = pkc[:, 1920:1928]

    # ---------- routing (both samples) ----------
    rwcols = []
    for b in range(BL):
        psA = spool.tile([128, 1], F32, tag="psA")
        nc.vector.tensor_reduce(psA, xp[b][:, 0:64, :], AX.XY, ALU.add)
        pparts = spool.tile([128, 14], F32, tag="pparts")
        for i in range(13):
            r0 = 64 + 5 * i
            rn = min(5, H + 2 - r0)
            pscr = fpool.tile([128, 5, W + 2], BF16, tag="pscr")
            nc.scalar.activation(
                pscr[:, 0:rn, :], xp[b][:, r0:r0 + rn, :], ACTF.Copy,
                accum_out=pparts[:, i:i + 1])
        nc.vector.memset(pparts[:, 13:14], 0.0)
        psB = spool.tile([128, 1], F32, tag="psB")
        nc.vector.tensor_reduce(psB, pparts, AX.X, ALU.add)
        psum_t = spool.tile([128, 1], F32, tag="psum_t")
        nc.vector.tensor_add(psum_t, psA, psB)

        mm1 = pm.tile([16, 1], F32, tag="m")
        nc.tensor.matmul(mm1, rw1t, psum_t, start=True, stop=True)
        h1 = spool.tile([16, 1], F32, tag="h1")
        nc.scalar.activation(h1, mm1, ACTF.Relu, bias=bb1, scale=gs1)
        mm2 = pm.tile([128, 1], F32, tag="m")
        nc.tensor.matmul(mm2, rw2t, h1, start=True, stop=True)
        gg = spool.tile([128, 1], F32, tag="gg")
        nc.scalar.activation(gg, mm2, ACTF.Sigmoid, bias=bb2, scale=gs2)
        mm3 = pm.tile([1, E], F32, tag="m")
        nc.tensor.matmul(mm3, gg, rw3t, start=True, stop=True)
        lg = spool.tile([1, E], F32, tag="lg")
        nc.vector.tensor_add(lg, mm3, rb3r)
        mx = spool.tile([1, 1], F32, tag="mx")
        nc.vector.tensor_reduce(mx, lg, AX.X, ALU.max)
        mxn = spool.tile([1, 1], F32, tag="mxn")
        nc.vector.tensor_scalar_mul(mxn, mx, -1.0)
        e16 = spool.tile([1, E], F32, tag="e16")
        nc.scalar.activation(e16, lg, ACTF.Exp, bias=mxn, scale=1.0)
        s1 = spool.tile([1, 1], F32, tag="s1")
        nc.vector.tensor_reduce(s1, e16, AX.X, ALU.add)
        rinv = spool.tile([1, 1], F32, tag="rinv")
        nc.vector.reciprocal(rinv, s1)
        rwrow = spool.tile([1, E], F32, tag="rwrow")
        nc.vector.tensor_scalar_mul(rwrow, e16, rinv)
        nc.sync.dma_start(srw_d[b].unsqueeze(0), rwrow)
        rwcol = spool.tile([128, 1], F32, tag=f"rwcol{b}", name=f"rwcol{b}")
        nc.sync.dma_start(rwcol, srw_d[b].unsqueeze(0).broadcast_to([8, E]))
        rwcols.append(rwcol)

    rwblk = spool.tile([128, 2 * RR], BF16, tag="rwblk")
    nc.vector.tensor_scalar_mul(rwblk[:, 0:8], bmask, rwcols[0])
    nc.vector.tensor_scalar_mul(rwblk[:, 8:16], bmask, rwcols[1])

    # ---------- wgen (both samples) ----------
    wsb = [wpool.tile([128, 9, 128], BF16, tag=f"wsb{b}", name=f"wsb{b}")
           for b in range(BL)]
    for og in range(16):
        ec = ecs[og]
        pw = pm.tile([128, 9, 16], F32, tag="m")
        for k in range(9):
            nc.tensor.matmul(pw[:, k, :], ec[:, k, :], rwblk,
                             start=True, stop=True)
        if og % 2 == 0:
            nc.scalar.activation(wsb[0][:, :, og * 8:og * 8 + 8],
                                 pw[:, :, 0:8], ACTF.Copy)
            nc.vector.tensor_copy(wsb[1][:, :, og * 8:og * 8 + 8],
                                  pw[:, :, 8:16])
        else:
            nc.vector.tensor_copy(wsb[0][:, :, og * 8:og * 8 + 8],
                                  pw[:, :, 0:8])
            nc.scalar.activation(wsb[1][:, :, og * 8:og * 8 + 8],
                                 pw[:, :, 8:16], ACTF.Copy)

    # CBAM sp-map tiles: pads written once, reused across samples
    spsum_t = spool.tile([128, 134], BF16, tag="spsum_t")   # [h, w+pad]
    spmax_wh = spool.tile([128, 134], BF16, tag="spmax_wh")  # [w, h+pad]
    for t in (spsum_t, spmax_wh):
        nc.vector.memset(t[:, 0:3], 0.0)
        nc.vector.memset(t[:, 131:134], 0.0)

    # ---------- per-sample phases, with sample-0 stats interleaved ----------
    osb = [opool.tile([128, H, W], BF16, tag="osb", name=f"osb{b}")
           for b in range(BL)]
    cw = [None, None]
    cwb = [None, None]

    def conv_strip(b, sup):
        pcs = [pc.tile([128, 4, W], F32, tag="c", name=f"pc{b}_{sup}_{g}")
               for g in range(2)]
        for k in range(9):
            kh, kw = divmod(k, 3)
            lhs = wsb[b][:, k, :]
            for g in range(2):
                r0 = sup * 8 + g * 4 + kh
                nc.tensor.matmul(pcs[g], lhs, xp[b][:, r0:r0 + 4, kw:kw + W],
                                 start=(k == 0), stop=(k == 8))
        for g in range(2):
            hr = sup * 8 + g * 4
            nc.scalar.activation(
                osb[b][:, hr:hr + 4, :], pcs[g], ACTF.Copy,
                accum_out=cparts[b][:, sup * 2 + g:sup * 2 + g + 1])

    cparts = [spool.tile([128, 32], F32, tag="cparts", name=f"cparts{b}")
              for b in range(BL)]
    diagcw = [None, None]

    def se_block(b):
        cps = spool.tile([128, 1], F32, tag="cps")
        nc.vector.tensor_reduce(cps, cparts[b], AX.X, ALU.add)
        se1 = pm.tile([16, 1], F32, tag="m")
        nc.tensor.matmul(se1, caw1t, cps, start=True, stop=True)
        chs = spool.tile([16, 1], F32, tag="chs")
        nc.scalar.activation(chs, se1, ACTF.Relu, bias=bbca1, scale=gsca1)
        se2 = pm.tile([128, 1], F32, tag="m")
        nc.tensor.matmul(se2, caw2t, chs, start=True, stop=True)
        cw[b] = spool.tile([128, 1], F32, tag=f"cw{b}", name=f"cw{b}")
        nc.scalar.activation(cw[b], se2, ACTF.Sigmoid, bias=bbca2, scale=gsca2)
        cwb[b] = spool.tile([128, 1], BF16, tag=f"cwb{b}", name=f"cwb{b}")
        nc.vector.tensor_copy(cwb[b], cw[b])
        diagcw[b] = spool.tile([128, 128], BF16, tag=f"diagcw{b}",
                               name=f"diagcw{b}")
        nc.vector.tensor_scalar_mul(diagcw[b], identb, cw[b])

    def stats_chunk(b, c):
        mf = fpool.tile([1, 4096], BF16, tag="mf")
        for j in range(8):
            h0 = c * 32 + j * 4
            ptt = pc.tile([128, 4, 128], F32, tag="c", name=f"ptt{b}_{c}_{j}")
            for i in range(4):
                nc.tensor.matmul(ptt[:, i, :], osb[b][:, h0 + i, :], diagcw[b],
                                 start=True, stop=True)
            pmean = pm.tile([1, 512], F32, tag="m")
            nc.tensor.matmul(pmean, cwb[b], osb[b][:, h0:h0 + 4, :],
                             start=True, stop=True)
            nc.vector.tensor_reduce(spmax_wh[:, 3 + h0:3 + h0 + 4], ptt,
                                    AX.X, ALU.max)
            nc.scalar.activation(mf[:, j * 512:(j + 1) * 512], pmean,
                                 ACTF.Copy)
        nc.sync.dma_start(spsum_t[c * 32:(c + 1) * 32, 3:131], mf)

    def banded_final(b):
        pswW = pm.tile([128, 128], F32, tag="m")
        for t in range(7):
            nc.tensor.matmul(pswW, mmax[:, t, :], spmax_wh[:, t:t + 128],
                             start=(t == 0), stop=(t == 6))
        swW = spool.tile([128, 128], BF16, tag="swW")
        nc.scalar.activation(swW, pswW, ACTF.Copy)
        psw = pm.tile([128, 128], F32, tag="m")
        for t in range(7):
            nc.tensor.matmul(psw, msum[:, t, :], spsum_t[:, t:t + 128],
                             start=(t == 0), stop=False)
        nc.tensor.matmul(psw, swW, identb, start=False, stop=True)
        swsb = spool.tile([128, 128], BF16, tag="swsb")
        nc.scalar.activation(swsb, psw, ACTF.Sigmoid, bias=bssa, scale=gssa)
        nc.sync.dma_start(ssw_d[b].rearrange("(h w) -> h w", h=128), swsb)

        for q in range(4):
            swbc = fpool.tile([128, 32, 128], BF16, tag="swbc")
            nc.sync.dma_start(
                swbc,
                ssw_d[b, q * 4096:(q + 1) * 4096].unsqueeze(0)
                .partition_broadcast(128))
            sl = osb[b][:, q * 32:(q + 1) * 32, :]
            nc.vector.tensor_mul(sl, sl, swbc)
            nc.vector.scalar_tensor_tensor(
                sl, sl, cw[b], xp[b][:, 1 + q * 32:33 + q * 32, 1:W + 1],
                ALU.mult, ALU.add)
            nc.scalar.dma_start(out_d[b, :, q * 32:(q + 1) * 32, :], sl)

    # sample 0 conv
    for sup in range(16):
        conv_strip(0, sup)
    se_block(0)
    # sample 1 conv with sample-0 stats interleaved
    for sup in range(16):
        conv_strip(1, sup)
        if sup % 2 == 1 and sup < 8:
            stats_chunk(0, sup // 2)
        if sup == 9:
            banded_final(0)
    se_block(1)
    for c in range(4):
        stats_chunk(1, c)
    banded_final(1)


def _host_prep(inp):
    import ml_dtypes
    experts = np.ascontiguousarray(inp["experts"], dtype=np.float32)
    ew = experts.reshape(E, 16, 8, CI, 9)          # [e, og, o', i, k]
    ew = ew.transpose(1, 2, 0, 4, 3)               # [og, o', e, k, i]
    ew = np.ascontiguousarray(ew).reshape(16, 128, 9, 128)

    bm = np.zeros((8, 16, 8), dtype=np.float32)
    for j in range(8):
        bm[j, :, j] = 1.0
    bm = bm.reshape(128, 8)

    saw = np.asarray(inp["sa_w"], np.float32).reshape(2, 7, 7)
    Ms = np.zeros((7, 128, 128), dtype=np.float32)
    Mm = np.zeros((7, 128, 128), dtype=np.float32)
    hp = np.arange(128)
    for dh in range(7):
        for dw in range(7):
            src = hp + dh - 3
            v = (src >= 0) & (src < 128)
            Ms[dw, src[v], hp[v]] += saw[0, dh, dw] / CO
            src2 = hp + dw - 3
            v2 = (src2 >= 0) & (src2 < 128)
            Mm[dh, src2[v2], hp[v2]] += saw[1, dh, dw]
    msum = np.ascontiguousarray(Ms.transpose(1, 0, 2)).reshape(128, 896)
    mmax = np.ascontiguousarray(Mm.transpose(1, 0, 2)).reshape(128, 896)

    # packa [128, 54] f32
    pka = np.zeros((128, 54), dtype=np.float32)
    pka[:, 0:16] = inp["rw1"].T
    pka[:, 16:32] = inp["rw3"].T
    pka[:, 32:48] = inp["ca_w1"].T
    pka[:, 48] = np.asarray(inp["rbn2_g"], np.float32) * BNS
    pka[:, 49] = np.asarray(inp["rbn2_b"], np.float32)
    pka[:, 50] = np.asarray(inp["ca_bn2_g"], np.float32) * BNS
    pka[:, 51] = np.asarray(inp["ca_bn2_b"], np.float32)
    pka[:, 52] = float(inp["sa_bn_g"][0]) * BNS
    pka[:, 53] = float(inp["sa_bn_b"][0])

    # packb [16, 260] f32
    pkb = np.zeros((16, 260), dtype=np.float32)
    pkb[:, 0:128] = inp["rw2"].T
    pkb[:, 128:256] = inp["ca_w2"].T
    pkb[:, 256] = np.asarray(inp["rbn1_g"], np.float32) * (BNS / HW)
    pkb[:, 257] = np.asarray(inp["rbn1_b"], np.float32)
    pkb[:, 258] = np.asarray(inp["ca_bn1_g"], np.float32) * (BNS / HW)
    pkb[:, 259] = np.asarray(inp["ca_bn1_b"], np.float32)

    # packc [128, 1928] bf16
    pkc = np.zeros((128, 1928), dtype=np.float32)
    pkc[:, 0:896] = msum
    pkc[:, 896:1792] = mmax
    pkc[:, 1792:1920] = np.eye(128, dtype=np.float32)
    pkc[:, 1920:1928] = bm

    x = np.asarray(inp["x"], np.float32)
    xpad = np.zeros((B, CI, H + 2, W + 2), dtype=ml_dtypes.bfloat16)
    xpad[:, :, 1:H + 1, 1:W + 1] = x

    shared = {
        "experts_w": ew.astype(ml_dtypes.bfloat16),
        "packa": pka,
        "packb": pkb,
        "packc": pkc.astype(ml_dtypes.bfloat16),
        "rb3": np.asarray(inp["rb3"], np.float32),
    }
    in_maps = []
    for c in range(NCORES):
        m = dict(shared)
        m["xpad"] = np.ascontiguousarray(xpad[BL * c:BL * (c + 1)])
        in_maps.append(m)
    return in_maps


def get_module():
    if "nc" not in _CACHE:
        _CACHE["nc"] = _build_module()
    return _CACHE["nc"]


def kernel(**inputs):
    nc = get_module()
    in_maps = _host_prep(inputs)
    res = run_bass_kernel_spmd(nc, in_maps, core_ids=list(range(NCORES)))
    out = np.concatenate([r["out"] for r in res.results], axis=0)
    return out.astype(np.float32)


# revision 7
# speedup vs baseline: 1.8858x; 1.0349x over previous
"""Trainium2 Bass kernel for EnhancedCondConv2d (moe_routing).

Data-parallel over batch: 8 cores x 2 samples each. Full inputs in,
full outputs back.

Per-core program:
  1. routing (both samples): avgpool(x) -> tiny MLP -> softmax
  2. wgen (both samples fused): w[b] = sum_e rw[e]*experts[e] as 144
     block-diag matmuls with contiguous (FWL-friendly) expert layout
  3. conv per sample: 9 PSUM-accumulated shifted matmuls per 8-row
     strip, double-buffered PSUM so PE never stalls on eviction
  4. SE channel attention folded into PSUM eviction accumulators
  5. CBAM stats: cw-scaled transpose of each h-plane via one matmul
     against diag(cw) -> DVE max-reduce gives the channel max in
     [w-part, h] layout; channel mean via cw-weighted column matmul in
     flat [h, w] layout. 7x7 conv as banded-Toeplitz matmuls in both
     layouts, folded into one PSUM via an identity matmul. Sample 0's
     stats interleave into sample 1's conv emission so PE stays dense.
  6. final: out = (osb*sw)*cw + x in place, all bf16 (host casts f32)
"""

import math
from contextlib import ExitStack

import numpy as np

import concourse.bass as bass
import concourse.bacc as bacc
import concourse.mybir as mybir
import concourse.tile as tile
from concourse.bass_utils import run_bass_kernel_spmd

F32 = mybir.dt.float32
BF16 = mybir.dt.bfloat16
AX = mybir.AxisListType
ALU = mybir.AluOpType
ACTF = mybir.ActivationFunctionType

B, CI, CO, H, W, E, RR = 16, 128, 128, 128, 128, 16, 8
NCORES = 8
BL = B // NCORES  # 2 samples per core
EPS = 1e-5
HW = H * W
BNS = 1.0 / math.sqrt(1.0 + EPS)

_CACHE = {}


def _build_module():
    nc = bacc.Bacc("TRN2", target_bir_lowering=False, debug=False)

    xp_d = nc.dram_tensor("xpad", [BL, CI, H + 2, W + 2], BF16,
                          kind="ExternalInput").ap()
    ew_d = nc.dram_tensor("experts_w", [16, 128, 9, 128], BF16,
                          kind="ExternalInput").ap()
    pka_d = nc.dram_tensor("packa", [128, 54], F32, kind="ExternalInput").ap()
    pkb_d = nc.dram_tensor("packb", [16, 260], F32, kind="ExternalInput").ap()
    pkc_d = nc.dram_tensor("packc", [128, 1928], BF16,
                           kind="ExternalInput").ap()
    rb3_d = nc.dram_tensor("rb3", [E], F32, kind="ExternalInput").ap()

    out_d = nc.dram_tensor("out", [BL, CO, H, W], BF16,
                           kind="ExternalOutput").ap()

    ssw_d = nc.dram_tensor("scr_sw", [BL, HW], BF16).ap()

    with tile.TileContext(nc) as tc, ExitStack() as ctx:
        _kernel_body(ctx, tc, xp_d, ew_d, pka_d, pkb_d, pkc_d, rb3_d,
                     out_d, ssw_d)
    nc.compile()
    return nc


def _kernel_body(ctx, tc, xp_d, ew_d, pka_d, pkb_d, pkc_d, rb3_d,
                 out_d, ssw_d):
    nc = tc.nc

    cpool = ctx.enter_context(tc.tile_pool(name="const", bufs=1))
    xpool = ctx.enter_context(tc.tile_pool(name="xp", bufs=1))
    opool = ctx.enter_context(tc.tile_pool(name="ob", bufs=2))
    wpool = ctx.enter_context(tc.tile_pool(name="wp", bufs=1))
    epool = ctx.enter_context(tc.tile_pool(name="ep", bufs=4))
    spool = ctx.enter_context(tc.tile_pool(name="sp", bufs=2))
    fpool = ctx.enter_context(tc.tile_pool(name="fp", bufs=2))

    pc = ctx.enter_context(tc.tile_pool(name="pc", bufs=4, space="PSUM"))
    pm = ctx.enter_context(tc.tile_pool(name="pm", bufs=4, space="PSUM"))

    # ---------- bulk loads in priority order, all on the sync ring ----------
    xp0 = xpool.tile([128, H + 2, W + 2], BF16, tag="xp0")
    nc.sync.dma_start(xp0[:, 0:66, :], xp_d[0, :, 0:66, :])
    nc.sync.dma_start(xp0[:, 66:130, :], xp_d[0, :, 66:130, :])

    pka = cpool.tile([128, 54], F32, tag="pka")
    nc.sync.dma_start(pka, pka_d)
    pkb = cpool.tile([16, 260], F32, tag="pkb")
    nc.sync.dma_start(pkb, pkb_d)
    rb3r = cpool.tile([1, E], F32, tag="rb3r")
    nc.sync.dma_start(rb3r, rb3_d.unsqueeze(0))

    xp1 = xpool.tile([128, H + 2, W + 2], BF16, tag="xp1")
    nc.sync.dma_start(xp1[:, 0:66, :], xp_d[1, :, 0:66, :])
    nc.sync.dma_start(xp1[:, 66:130, :], xp_d[1, :, 66:130, :])
    xp = [xp0, xp1]

    pkc = cpool.tile([128, 1928], BF16, tag="pkc")
    nc.sync.dma_start(pkc, pkc_d)

    one11 = cpool.tile([1, 1], F32, tag="one11")
    nc.vector.memset(one11, 1.0)

    ecs = []
    for og in range(16):
        ec = epool.tile([128, 9, 128], BF16, tag="ec", name=f"ec{og}")
        nc.sync.dma_start(ec, ew_d[og])
        ecs.append(ec)

    # const views
    rw1t = pka[:, 0:16]
    rw3t = pka[:, 16:32]
    caw1t = pka[:, 32:48]
    gs2 = pka[:, 48:49]
    bb2 = pka[:, 49:50]
    gsca2 = pka[:, 50:51]
    bbca2 = pka[:, 51:52]
    gssa = pka[:, 52:53]
    bssa = pka[:, 53:54]
    rw2t = pkb[:, 0:128]
    caw2t = pkb[:, 128:256]
    gs1 = pkb[:, 256:257]
    bb1 = pkb[:, 257:258]
    gsca1 = pkb[:, 258:259]
    bbca1 = pkb[:, 259:260]
    msum = pkc[:, 0:896].rearrange("p (t i) -> p t i", t=7)
    mmax = pkc[:, 896:1792].rearrange("p (t i) -> p t i", t=7)
    identb = pkc[:, 1792:1920]
    bmask = pkc[:, 1920:1928]

    # ---------- routing (both samples) ----------
    rwcols = []
    for b in range(BL):
        psA = spool.tile([128, 1], F32, tag="psA")
        nc.vector.tensor_reduce(psA, xp[b][:, 0:64, :], AX.XY, ALU.add)
        pparts = spool.tile([128, 14], F32, tag="pparts")
        for i in range(13):
            r0 = 64 + 5 * i
            rn = min(5, H + 2 - r0)
            pscr = fpool.tile([128, 5, W + 2], BF16, tag="pscr")
            nc.scalar.activation(
                pscr[:, 0:rn, :], xp[b][:, r0:r0 + rn, :], ACTF.Copy,
                accum_out=pparts[:, i:i + 1])
        nc.vector.memset(pparts[:, 13:14], 0.0)
        psB = spool.tile([128, 1], F32, tag="psB")
        nc.vector.tensor_reduce(psB, pparts, AX.X, ALU.add)
        psum_t = spool.tile([128, 1], F32, tag="psum_t")
        nc.vector.tensor_add(psum_t, psA, psB)

        mm1 = pm.tile([16, 1], F32, tag="m")
        nc.tensor.matmul(mm1, rw1t, psum_t, start=True, stop=True)
        h1 = spool.tile([16, 1], F32, tag="h1")
        nc.scalar.activation(h1, mm1, ACTF.Relu, bias=bb1, scale=gs1)
        mm2 = pm.tile([128, 1], F32, tag="m")
        nc.tensor.matmul(mm2, rw2t, h1, start=True, stop=True)
        gg = spool.tile([128, 1], F32, tag="gg")
        nc.scalar.activation(gg, mm2, ACTF.Sigmoid, bias=bb2, scale=gs2)
        mm3 = pm.tile([1, E], F32, tag="m")
        nc.tensor.matmul(mm3, gg, rw3t, start=True, stop=True)
        lg = spool.tile([1, E], F32, tag="lg")
        nc.vector.tensor_add(lg, mm3, rb3r)
        mx = spool.tile([1, 1], F32, tag="mx")
        nc.vector.tensor_reduce(mx, lg, AX.X, ALU.max)
        mxn = spool.tile([1, 1], F32, tag="mxn")
        nc.vector.tensor_scalar_mul(mxn, mx, -1.0)
        e16 = spool.tile([1, E], F32, tag="e16")
        nc.scalar.activation(e16, lg, ACTF.Exp, bias=mxn, scale=1.0)
        s1 = spool.tile([1, 1], F32, tag="s1")
        nc.vector.tensor_reduce(s1, e16, AX.X, ALU.add)
        rinv = spool.tile([1, 1], F32, tag="rinv")
        nc.vector.reciprocal(rinv, s1)
        e128 = spool.tile([1, 128], F32, tag="e128")
        nc.vector.tensor_scalar_mul(
            e128.rearrange("p (a c) -> p a c", a=8),
            e16.unsqueeze(1).broadcast_to([1, 8, E]), rinv)
        pcol = pm.tile([128, 1], F32, tag="m")
        nc.tensor.matmul(pcol, e128, one11, start=True, stop=True)
        rwcol = spool.tile([128, 1], F32, tag=f"rwcol{b}", name=f"rwcol{b}")
        nc.vector.tensor_copy(rwcol, pcol)
        rwcols.append(rwcol)

    rwblk = spool.tile([128, 2 * RR], BF16, tag="rwblk")
    nc.vector.tensor_scalar_mul(rwblk[:, 0:8], bmask, rwcols[0])
    nc.vector.tensor_scalar_mul(rwblk[:, 8:16], bmask, rwcols[1])

    # ---------- wgen (both samples) ----------
    wsb = [wpool.tile([128, 9, 128], BF16, tag=f"wsb{b}", name=f"wsb{b}")
           for b in range(BL)]
    for og in range(16):
        ec = ecs[og]
        pw = pm.tile([128, 9, 16], F32, tag="m")
        for k in range(9):
            nc.tensor.matmul(pw[:, k, :], ec[:, k, :], rwblk,
                             start=True, stop=True)
        if og % 2 == 0:
            nc.scalar.activation(wsb[0][:, :, og * 8:og * 8 + 8],
                                 pw[:, :, 0:8], ACTF.Copy)
            nc.vector.tensor_copy(wsb[1][:, :, og * 8:og * 8 + 8],
                                  pw[:, :, 8:16])
        else:
            nc.vector.tensor_copy(wsb[0][:, :, og * 8:og * 8 + 8],
                                  pw[:, :, 0:8])
            nc.scalar.activation(wsb[1][:, :, og * 8:og * 8 + 8],
                                 pw[:, :, 8:16], ACTF.Copy)

    # CBAM sp-map tiles: pads written once, reused across samples
    spsum_t = spool.tile([128, 134], BF16, tag="spsum_t")   # [h, w+pad]
    spmax_wh = spool.tile([128, 134], BF16, tag="spmax_wh")  # [w, h+pad]
    for t in (spsum_t, spmax_wh):
        nc.vector.memset(t[:, 0:3], 0.0)
        nc.vector.memset(t[:, 131:134], 0.0)

    # ---------- per-sample phases, with sample-0 stats interleaved ----------
    osb = [opool.tile([128, H, W], BF16, tag="osb", name=f"osb{b}")
           for b in range(BL)]
    cw = [None, None]
    cwb = [None, None]

    def conv_strip(b, sup):
        pcs = [pc.tile([128, 4, W], F32, tag="c", name=f"pc{b}_{sup}_{g}")
               for g in range(2)]
        for k in range(9):
            kh, kw = divmod(k, 3)
            lhs = wsb[b][:, k, :]
            for g in range(2):
                r0 = sup * 8 + g * 4 + kh
                nc.tensor.matmul(pcs[g], lhs, xp[b][:, r0:r0 + 4, kw:kw + W],
                                 start=(k == 0), stop=(k == 8))
        for g in range(2):
            hr = sup * 8 + g * 4
            nc.scalar.activation(
                osb[b][:, hr:hr + 4, :], pcs[g], ACTF.Copy,
                accum_out=cparts[b][:, sup * 2 + g:sup * 2 + g + 1])

    cparts = [spool.tile([128, 32], F32, tag="cparts", name=f"cparts{b}")
              for b in range(BL)]
    diagcw = [None, None]

    def se_block(b):
        cps = spool.tile([128, 1], F32, tag="cps")
        nc.vector.tensor_reduce(cps, cparts[b], AX.X, ALU.add)
        se1 = pm.tile([16, 1], F32, tag="m")
        nc.tensor.matmul(se1, caw1t, cps, start=True, stop=True)
        chs = spool.tile([16, 1], F32, tag="chs")
        nc.scalar.activation(chs, se1, ACTF.Relu, bias=bbca1, scale=gsca1)
        se2 = pm.tile([128, 1], F32, tag="m")
        nc.tensor.matmul(se2, caw2t, chs, start=True, stop=True)
        cw[b] = spool.tile([128, 1], F32, tag=f"cw{b}", name=f"cw{b}")
        nc.scalar.activation(cw[b], se2, ACTF.Sigmoid, bias=bbca2, scale=gsca2)
        cwb[b] = spool.tile([128, 1], BF16, tag=f"cwb{b}", name=f"cwb{b}")
        nc.vector.tensor_copy(cwb[b], cw[b])
        diagcw[b] = spool.tile([128, 128], BF16, tag=f"diagcw{b}",
                               name=f"diagcw{b}")
        nc.vector.tensor_scalar_mul(diagcw[b], identb, cw[b])

    def stats_chunk(b, c):
        mf = fpool.tile([1, 4096], BF16, tag="mf")
        for j in range(8):
            h0 = c * 32 + j * 4
            ptt = pc.tile([128, 4, 128], F32, tag="c", name=f"ptt{b}_{c}_{j}")
            for i in range(4):
                nc.tensor.matmul(ptt[:, i, :], osb[b][:, h0 + i, :], diagcw[b],
                                 start=True, stop=True)
            pmean = pm.tile([1, 512], F32, tag="m")
            nc.tensor.matmul(pmean, cwb[b], osb[b][:, h0:h0 + 4, :],
                             start=True, stop=True)
            nc.vector.tensor_reduce(spmax_wh[:, 3 + h0:3 + h0 + 4], ptt,
                                    AX.X, ALU.max)
            nc.scalar.activation(mf[:, j * 512:(j + 1) * 512], pmean,
                                 ACTF.Copy)
        nc.sync.dma_start(spsum_t[c * 32:(c + 1) * 32, 3:131], mf)

    def banded_final(b):
        for c4 in range(4):
            sc = osb[b][:, c4 * 32:(c4 + 1) * 32, :]
            nc.vector.tensor_scalar_mul(sc, sc, cw[b])
        pswW = pm.tile([128, 128], F32, tag="m")
        for t in range(7):
            nc.tensor.matmul(pswW, mmax[:, t, :], spmax_wh[:, t:t + 128],
                             start=(t == 0), stop=(t == 6))
        swW = spool.tile([128, 128], BF16, tag="swW")
        nc.scalar.activation(swW, pswW, ACTF.Copy)
        psw = pm.tile([128, 128], F32, tag="m")
        for t in range(7):
            nc.tensor.matmul(psw, msum[:, t, :], spsum_t[:, t:t + 128],
                             start=(t == 0), stop=False)
        nc.tensor.matmul(psw, swW, identb, start=False, stop=True)
        swsb = spool.tile([128, 128], BF16, tag="swsb")
        nc.scalar.activation(swsb, psw, ACTF.Sigmoid, bias=bssa, scale=gssa)
        nc.sync.dma_start(ssw_d[b].rearrange("(h w) -> h w", h=128), swsb)

        for q in range(4):
            swbc = fpool.tile([128, 32, 128], BF16, tag="swbc")
            nc.sync.dma_start(
                swbc,
                ssw_d[b, q * 4096:(q + 1) * 4096].unsqueeze(0)
                .partition_broadcast(128))
            sl = osb[b][:, q * 32:(q + 1) * 32, :]
            nc.vector.tensor_mul(sl, sl, swbc)
            nc.vector.tensor_add(
                sl, sl, xp[b][:, 1 + q * 32:33 + q * 32, 1:W + 1])
            nc.scalar.dma_start(out_d[b, :, q * 32:(q + 1) * 32, :], sl)

    # sample 0 conv
    for sup in range(16):
        conv_strip(0, sup)
    se_block(0)
    # sample 1 conv with sample-0 stats interleaved
    for sup in range(16):
        conv_strip(1, sup)
        if sup % 2 == 1 and sup < 8:
            stats_chunk(0, sup // 2)
        if sup == 9:
            banded_final(0)
    se_block(1)
    for c in range(4):
        stats_chunk(1, c)
    banded_final(1)


def _host_prep(inp):
    import ml_dtypes
    experts = np.ascontiguousarray(inp["experts"], dtype=np.float32)
    ew = experts.reshape(E, 16, 8, CI, 9)          # [e, og, o', i, k]
    ew = ew.transpose(1, 2, 0, 4, 3)               # [og, o', e, k, i]
    ew = np.ascontiguousarray(ew).reshape(16, 128, 9, 128)

    bm = np.zeros((8, 16, 8), dtype=np.float32)
    for j in range(8):
        bm[j, :, j] = 1.0
    bm = bm.reshape(128, 8)

    saw = np.asarray(inp["sa_w"], np.float32).reshape(2, 7, 7)
    Ms = np.zeros((7, 128, 128), dtype=np.float32)
    Mm = np.zeros((7, 128, 128), dtype=np.float32)
    hp = np.arange(128)
    for dh in range(7):
        for dw in range(7):
            src = hp + dh - 3
            v = (src >= 0) & (src < 128)
            Ms[dw, src[v], hp[v]] += saw[0, dh, dw] / CO
            src2 = hp + dw - 3
            v2 = (src2 >= 0) & (src2 < 128)
            Mm[dh, src2[v2], hp[v2]] += saw[1, dh, dw]
    msum = np.ascontiguousarray(Ms.transpose(1, 0, 2)).reshape(128, 896)
    mmax = np.ascontiguousarray(Mm.transpose(1, 0, 2)).reshape(128, 896)

    # packa [128, 54] f32
    pka = np.zeros((128, 54), dtype=np.float32)
    pka[:, 0:16] = inp["rw1"].T
    pka[:, 16:32] = inp["rw3"].T
    pka[:, 32:48] = inp["ca_w1"].T
    pka[:, 48] = np.asarray(inp["rbn2_g"], np.float32) * BNS
    pka[:, 49] = np.asarray(inp["rbn2_b"], np.float32)
    pka[:, 50] = np.asarray(inp["ca_bn2_g"], np.float32) * BNS
    pka[:, 51] = np.asarray(inp["ca_bn2_b"], np.float32)
    pka[:, 52] = float(inp["sa_bn_g"][0]) * BNS
    pka[:, 53] = float(inp["sa_bn_b"][0])

    # packb [16, 260] f32
    pkb = np.zeros((16, 260), dtype=np.float32)
    pkb[:, 0:128] = inp["rw2"].T
    pkb[:, 128:256] = inp["ca_w2"].T
    pkb[:, 256] = np.asarray(inp["rbn1_g"], np.float32) * (BNS / HW)
    pkb[:, 257] = np.asarray(inp["rbn1_b"], np.float32)
    pkb[:, 258] = np.asarray(inp["ca_bn1_g"], np.float32) * (BNS / HW)
    pkb[:, 259] = np.asarray(inp["ca_bn1_b"], np.float32)

    # packc [128, 1928] bf16
    pkc = np.zeros((128, 1928), dtype=np.float32)
    pkc[:, 0:896] = msum
    pkc[:, 896:1792] = mmax
    pkc[:, 1792:1920] = np.eye(128, dtype=np.float32)
    pkc[:, 1920:1928] = bm

    x = np.asarray(inp["x"], np.float32)
    xpad = np.zeros((B, CI, H + 2, W + 2), dtype=ml_dtypes.bfloat16)
    xpad[:, :, 1:H + 1, 1:W + 1] = x

    shared = {
        "experts_w": ew.astype(ml_dtypes.bfloat16),
        "packa": pka,
        "packb": pkb,
        "packc": pkc.astype(ml_dtypes.bfloat16),
        "rb3": np.asarray(inp["rb3"], np.float32),
    }
    in_maps = []
    for c in range(NCORES):
        m = dict(shared)
        m["xpad"] = np.ascontiguousarray(xpad[BL * c:BL * (c + 1)])
        in_maps.append(m)
    return in_maps


def get_module():
    if "nc" not in _CACHE:
        _CACHE["nc"] = _build_module()
    return _CACHE["nc"]


def kernel(**inputs):
    nc = get_module()
    in_maps = _host_prep(inputs)
    res = run_bass_kernel_spmd(nc, in_maps, core_ids=list(range(NCORES)))
    out = np.concatenate([r["out"] for r in res.results], axis=0)
    return out.astype(np.float32)
